# revision 1
# baseline (speedup 1.0000x reference)
"""MoE layer (shared expert + top-2 routed experts) on 8 NeuronCores.

Strategy (expert-parallel, routing-aware):
  - Router (softmax -> top-2 -> renorm) computed on host in float64; it is
    tiny (8192x8) and must match the reference's top-k selection.
  - Core c owns routed expert c: host gathers the tokens routed to expert c
    (~2k of 8192*2 assignments), pads to a common capacity C, and the device
    runs a dense SwiGLU MLP over just those tokens (bf16 matmuls, fp32 accum).
  - The shared expert is data-parallel: core c also runs the shared SwiGLU
    over tokens [c*1024, (c+1)*1024).
  - Combine is done on host: gate-scale each expert's token outputs and
    scatter-add; every token has exactly two routed contributions.

Device layout: activations are kept transposed ([d, tokens]) so the native
[K, M] weight layouts of ew1/ew2/ew3 feed nc.tensor.matmul directly with no
on-device transposes. All matmul inputs are bf16 (PE full rate + FWL),
accumulation is fp32 in PSUM, outputs are written back as bf16.

DMA layout: every dram parameter is packed on the host so that each
dma_start moves one fully-contiguous-per-partition block (4-16 KiB lines).
The baseline's strided layouts produced 1 KiB descriptor fragments, which
made the prologue descriptor-bound: the first real matmul started at
~14.2 us and the PE ran its first ~14 matmuls at the cold 1.2 GHz clock
(HAM re-throttle after a 3.5 us idle gap). Contiguous blocks + a right-
sized PE warmup keep the clock gate open and start the stream ~7 us
earlier. Routed chunks are near-equal (e.g. 5x~428 for C=2136) instead of
512x4+88: N>=128 keeps every chunk at the full weight-load-overlapped
issue rate.
"""

import sys

for _p in ("/opt/trn_rl_repo",):
    if _p not in sys.path:
        sys.path.append(_p)

import numpy as np
import ml_dtypes

import concourse.bass as bass  # noqa: F401  (engine types via nc)
import concourse.mybir as mybir
import concourse.tile as tile
from concourse import bacc
from concourse.bass_utils import run_bass_kernel_spmd

D = 1024
H = 2048
E = 8
N_TOK = 8192  # 4 * 2048
S = N_TOK // E  # shared-expert tokens per core
KD = D // 128  # 8  k-subtiles over d
KH = H // 128  # 16 k-subtiles over h
MH = H // 128  # 16 m-tiles over h
MD = D // 128  # 8  m-tiles over d
NCHUNK = 512
FIRST = 320  # first routed chunk: shorter critical-path DMA to first matmul
WARMUP = 60  # PE warmup matmuls issued while the DMA prologue runs

BF = mybir.dt.bfloat16
F32 = mybir.dt.float32

_program_cache: dict[int, tuple] = {}


def _chunk_sizes(total: int, cap: int = NCHUNK) -> list[int]:
    """Split `total` into near-equal chunks of at most `cap` tokens."""
    nch = max(1, -(-total // cap))
    base, rem = divmod(total, nch)
    return [base + (1 if i < rem else 0) for i in range(nch)]


class _WBlocks:
    """Weight tiles split into `ns` column blocks of `bc` columns each."""

    def __init__(self, tiles, ktiles, bc):
        self.tiles = tiles
        self.ktiles = ktiles
        self.bc = bc

    def slice(self, k, m):
        j, c0 = divmod(m * 128, self.bc)
        return self.tiles[j][:, k, c0 : c0 + 128]


def _emit_moe(nc, tc, pools, params):
    """Emit the whole per-core program: routed expert over the gathered
    tokens followed by the shared expert over S tokens. Both are SwiGLU MLPs
    on [d-part, token-free] activations. mm3 of each token-chunk is deferred
    by one chunk so the PE never waits on the DVE h-tile handoff; the first
    chunk of each phase runs all-u-then-all-v so the second weight matrix can
    still be in flight."""
    wpool, xpool, hpool, hspool, opool, ppool, popool = pools
    (w1_ds, w2_ds, w3_ds, s1_ds, s2_ds, s3_ds,
     xg_ds, xs_ds, outr_ds, outs_ds, rsz, ssz) = params

    def load_w(drams, ktiles, tag, n_act=0):
        """One dma_start per column block; each block is one contiguous
        per-partition span in DRAM. The first n_act blocks go down the
        Activation engine's HWDGE ring, which is idle ~1.5us before the
        Sync ring clears its preamble — earliest possible first bytes."""
        ts_ = []
        for j, dram in enumerate(drams):
            t = wpool.tile(list(dram.shape), BF, tag=f"{tag}{j}", name=f"{tag}{j}")
            nc.sync.dma_start(t[:], dram[:])
            ts_.append(t)
        return _WBlocks(ts_, ktiles, drams[0].shape[2])

    # chunk-0 tokens first, then w1: the first matmuls need only x0 and the
    # leading w1 block, so those bytes go down the HWDGE FIFO first.
    # x blocks are padded to the full 512-token tile in DRAM so the DMA is
    # contiguous on BOTH sides (8 KiB per-partition spans, not 856 B pieces);
    # chunk 0 gets a dedicated exact-size tile so its critical-path DMA
    # moves only the real tokens
    xc0 = xpool.tile([128, KD, rsz[0]], BF, tag="xc0")
    nc.sync.dma_start(xc0[:], xg_ds[0][:])
    # PE warmup on an initialized scratch tile while the DMA prologue runs:
    # opens the HAM clock gate so real matmuls start at (or quickly reach)
    # the full 2.4 GHz clock
    warm = wpool.tile([128, 128], BF, tag="warm")
    nc.vector.memset(warm[:], 0.0)
    pwarm = ppool.tile([128, NCHUNK], F32, tag="pu")
    for _ in range(WARMUP):
        nc.tensor.matmul(pwarm[:, :128], warm[:], warm[:], start=True, stop=True)
    wa = load_w(w1_ds, KD, "wa")
    wb = load_w(w2_ds, KD, "wb")
    xc1 = None
    if len(xg_ds) > 1:
        xc1 = xpool.tile([128, KD, NCHUNK], BF, tag="xc")
        nc.sync.dma_start(xc1[:], xg_ds[1][:])
    # w3 (needed at the first deferred mm3, ~60us in) and shared w1 (needed
    # at the shared phase, ~390us in) are loaded from inside the job loop:
    # the two NeuronCores on one HBM stack otherwise exceed the stack
    # bandwidth during the startup burst, which is what made slow-DMA
    # outlier cores
    wc = sa = sb = sc = None

    def emit_uv(W1, W2, xc, h, nsz, split):
        if split:
            for m in range(MH):
                pu = ppool.tile([128, NCHUNK], F32, tag="pu")
                for k in range(KD):
                    nc.tensor.matmul(
                        pu[:, :nsz],
                        W1.slice(k, m),
                        xc[:, k, :nsz],
                        start=(k == 0),
                        stop=(k == KD - 1),
                    )
                nc.scalar.activation(
                    h[:, m, :nsz], pu[:, :nsz], mybir.ActivationFunctionType.Silu
                )
            for m in range(MH):
                pv = ppool.tile([128, NCHUNK], F32, tag="pv")
                for k in range(KD):
                    nc.tensor.matmul(
                        pv[:, :nsz],
                        W2.slice(k, m),
                        xc[:, k, :nsz],
                        start=(k == 0),
                        stop=(k == KD - 1),
                    )
                nc.vector.tensor_mul(h[:, m, :nsz], h[:, m, :nsz], pv[:, :nsz])
        else:
            for m in range(MH):
                pu = ppool.tile([128, NCHUNK], F32, tag="pu")
                pv = ppool.tile([128, NCHUNK], F32, tag="pv")
                for k in range(KD):
                    nc.tensor.matmul(
                        pu[:, :nsz],
                        W1.slice(k, m),
                        xc[:, k, :nsz],
                        start=(k == 0),
                        stop=(k == KD - 1),
                    )
                for k in range(KD):
                    nc.tensor.matmul(
                        pv[:, :nsz],
                        W2.slice(k, m),
                        xc[:, k, :nsz],
                        start=(k == 0),
                        stop=(k == KD - 1),
                    )
                hs = hspool.tile([128, NCHUNK], F32, tag="hs")
                nc.scalar.activation(
                    hs[:, :nsz], pu[:, :nsz], mybir.ActivationFunctionType.Silu
                )
                nc.vector.tensor_mul(h[:, m, :nsz], hs[:, :nsz], pv[:, :nsz])

    def emit_mm3(W3, h, nsz, out_d, stream):
        ot = opool.tile([128, MD, NCHUNK], BF, tag="ot")
        for mo in range(MD):
            po = popool.tile([128, NCHUNK], F32, tag="po")
            for k in range(KH):
                nc.tensor.matmul(
                    po[:, :nsz],
                    W3.slice(k, mo),
                    h[:, k, :nsz],
                    start=(k == 0),
                    stop=(k == KH - 1),
                )
            nc.vector.tensor_copy(ot[:, mo, :nsz], po[:, :nsz])
            if stream:
                # final chunk: per-mo DMAs so the drain after the last
                # matmul is one small transfer, not the whole chunk
                nc.sync.dma_start(out_d[:, mo, :], ot[:, mo, :nsz])
        if not stream:
            nc.sync.dma_start(out_d[:], ot[:, :, :nsz])

    jobs = [("r", ni) for ni in range(len(xg_ds))]
    jobs += [("s", ni) for ni in range(len(xs_ds))]

    deferred = None
    for ji, (ph, ni) in enumerate(jobs):
        if ji == 1 and wc is None:
            wc = load_w(w3_ds, KH, "wc")
        if (ji == 2 or ph == "s") and sa is None:
            sa = load_w(s1_ds, KD, "sa")  # own slots; prefetch early
        if ph == "r":
            x_d, out_d, W1, W2 = xg_ds[ni], outr_ds[ni], wa, wb
        else:
            if sb is None:
                sb = load_w(s2_ds, KD, "wb")  # reuses w2 slots (WAR-ordered)
            x_d, out_d, W1, W2 = xs_ds[ni], outs_ds[ni], sa, sb
        nsz = (rsz if ph == "r" else ssz)[ni]
        if ji == 0:
            xc = xc0
        elif ji == 1 and xc1 is not None:
            xc = xc1
        else:
            xc = xpool.tile([128, KD, NCHUNK], BF, tag="xc")
            nc.sync.dma_start(xc[:], x_d[:])
        h = hpool.tile([128, KH, NCHUNK], BF, tag="h")
        emit_uv(W1, W2, xc, h, nsz, split=(ni == 0))
        if deferred is not None:
            dph, dh, dnsz, dout = deferred
            emit_mm3(wc if dph == "r" else sc, dh, dnsz, dout, stream=False)
            if ph == "s" and sc is None:
                sc = load_w(s3_ds, KH, "wc")  # reuses w3 slots (WAR-ordered)
        deferred = (ph, h, nsz, out_d)
    dph, dh, dnsz, dout = deferred
    emit_mm3(wc if dph == "r" else sc, dh, dnsz, dout, stream=True)


def _build_program(C: int):
    nc = bacc.Bacc(None, target_bir_lowering=False)

    # small-ish first chunk to start the stream early, near-equal rest;
    # FIRST=320 keeps the chunk-0 u/v phases long enough that the weight
    # FIFO stays ahead even on cores whose DMA starts late
    if C > FIRST + NCHUNK:
        rsz = [FIRST] + _chunk_sizes(C - FIRST)
    else:
        rsz = _chunk_sizes(C)
    ssz = _chunk_sizes(S)

    def wparams(name, ktiles, ns):
        # w1/w2 free dim is H, w3 free dim is D
        bc = (H if ktiles == KD else D) // ns
        return [
            nc.declare_dram_parameter(f"{name}_{j}", [128, ktiles, bc], BF,
                                      isOutput=False)
            for j in range(ns)
        ]

    w1_ds = wparams("w1", KD, 16)
    w2_ds = wparams("w2", KD, 4)
    w3_ds = wparams("w3", KH, 2)
    s1_ds = wparams("s1", KD, 2)
    s2_ds = wparams("s2", KD, 4)  # must match w2's block shape (slot reuse)
    s3_ds = wparams("s3", KH, 2)  # must match w3's block shape (slot reuse)
    # chunk 0 is exact-size (critical-path DMA); later chunks are padded to
    # the full tile so their DMAs stay contiguous on both sides
    xg_ds = [
        nc.declare_dram_parameter(
            f"xg{i}", [128, KD, sz if i == 0 else NCHUNK], BF, isOutput=False
        )
        for i, sz in enumerate(rsz)
    ]
    xs_ds = [
        nc.declare_dram_parameter(f"xs{i}", [128, KD, NCHUNK], BF, isOutput=False)
        for i, sz in enumerate(ssz)
    ]
    outr_ds = [
        nc.declare_dram_parameter(f"or{i}", [128, MD, sz], BF, isOutput=True)
        for i, sz in enumerate(rsz)
    ]
    outs_ds = [
        nc.declare_dram_parameter(f"os{i}", [128, MD, sz], BF, isOutput=True)
        for i, sz in enumerate(ssz)
    ]

    with tile.TileContext(nc) as tc:
        with (
            tc.tile_pool(name="wpool", bufs=1) as wpool,
            tc.tile_pool(name="xpool", bufs=2) as xpool,
            tc.tile_pool(name="hpool", bufs=2) as hpool,
            tc.tile_pool(name="hspool", bufs=2) as hspool,
            tc.tile_pool(name="opool", bufs=1) as opool,
            tc.tile_pool(name="ppool", bufs=3, space="PSUM") as ppool,
            tc.tile_pool(name="popool", bufs=2, space="PSUM") as popool,
        ):
            pools = (wpool, xpool, hpool, hspool, opool, ppool, popool)
            params = (w1_ds, w2_ds, w3_ds, s1_ds, s2_ds, s3_ds,
                      xg_ds, xs_ds, outr_ds, outs_ds, rsz, ssz)
            _emit_moe(nc, tc, pools, params)

    nc.compile()
    return nc, rsz, ssz


def _get_program(C: int):
    if C not in _program_cache:
        _program_cache[C] = _build_program(C)
    return _program_cache[C]


def _pack_x_chunk(a: np.ndarray, pad: bool) -> np.ndarray:
    """[sz, D] host activation -> [128, KD, sz|NCHUNK] bf16 device block,
    zero-padded to the full tile (except chunk 0) so the DMA is contiguous
    on both sides (d on partitions as d = ko*128 + pi, tokens free)."""
    sz = a.shape[0]
    if pad and sz < NCHUNK:
        a = np.concatenate([a, np.zeros((NCHUNK - sz, D), dtype=a.dtype)])
    b = a.reshape(a.shape[0], KD, 128).transpose(2, 1, 0)
    return np.ascontiguousarray(b.astype(ml_dtypes.bfloat16))


def _pack_w(w: np.ndarray, ns: int) -> list[np.ndarray]:
    """[K, M] host weight (contraction dim first) -> ns blocks of
    [128, K//128, M//ns] bf16, each contiguous per partition."""
    K, M = w.shape
    kk, bc = K // 128, M // ns
    arr = w.reshape(kk, 128, M)
    return [
        np.ascontiguousarray(
            arr[:, :, j * bc : (j + 1) * bc].transpose(1, 0, 2).astype(
                ml_dtypes.bfloat16
            )
        )
        for j in range(ns)
    ]


def _unpack_out(blocks: list[np.ndarray]) -> np.ndarray:
    """Per-chunk [128, MD, sz] bf16 device outputs -> [T, D] fp32 host."""
    outs = []
    for b in blocks:
        sz = b.shape[2]
        outs.append(
            np.asarray(b).astype(np.float32).transpose(2, 1, 0).reshape(sz, D)
        )
    return np.concatenate(outs, axis=0)


def kernel(x, sw1, sw2, sw3, ew1, ew2, ew3, rw, rb):
    x = np.asarray(x, dtype=np.float32)
    sw1, sw2, sw3 = (np.asarray(a, dtype=np.float32) for a in (sw1, sw2, sw3))
    ew1, ew2, ew3 = (np.asarray(a, dtype=np.float32) for a in (ew1, ew2, ew3))
    rw = np.asarray(rw, dtype=np.float32)
    rb = np.asarray(rb, dtype=np.float32)
    xf = np.ascontiguousarray(x.reshape(N_TOK, D), dtype=np.float32)

    # --- host router (float64 to track the fp32 reference's ordering) ---
    logits = xf.astype(np.float64) @ rw.astype(np.float64) + rb.astype(np.float64)
    logits -= logits.max(axis=1, keepdims=True)
    p = np.exp(logits)
    p /= p.sum(axis=1, keepdims=True)
    order = np.argsort(-p, axis=1, kind="stable")
    idx = order[:, :2]  # [N, 2] expert ids, top-2
    w = np.take_along_axis(p, idx, axis=1)
    w = w / w.sum(axis=1, keepdims=True)

    tok_lists = []
    gate_lists = []
    for e in range(E):
        sel = idx == e  # [N, 2]
        any_e = sel.any(axis=1)
        tok = np.nonzero(any_e)[0]
        ge = np.where(sel[tok, 0], w[tok, 0], w[tok, 1])
        tok_lists.append(tok)
        gate_lists.append(ge.astype(np.float64))

    maxT = max(len(t) for t in tok_lists)
    C = max(256, maxT)

    nc, rsz, ssz = _get_program(C)
    rofs = np.cumsum([0] + rsz)
    sofs = np.cumsum([0] + ssz)

    # --- per-core input maps ---
    w1s = [_pack_w(ew1[e], 16) for e in range(E)]
    w2s = [_pack_w(ew2[e], 4) for e in range(E)]
    w3s = [_pack_w(ew3[e], 2) for e in range(E)]
    s1 = _pack_w(sw1, 2)
    s2 = _pack_w(sw2, 4)
    s3 = _pack_w(sw3, 2)

    in_maps = []
    for e in range(E):
        tok = tok_lists[e]
        xg = np.zeros((C, D), dtype=np.float32)
        xg[: len(tok)] = xf[tok]
        m = {}
        for j, blk in enumerate(w1s[e]):
            m[f"w1_{j}"] = blk
        for j, blk in enumerate(w2s[e]):
            m[f"w2_{j}"] = blk
        for j, blk in enumerate(w3s[e]):
            m[f"w3_{j}"] = blk
        for j, blk in enumerate(s1):
            m[f"s1_{j}"] = blk
        for j, blk in enumerate(s2):
            m[f"s2_{j}"] = blk
        for j, blk in enumerate(s3):
            m[f"s3_{j}"] = blk
        for i, sz in enumerate(rsz):
            m[f"xg{i}"] = _pack_x_chunk(xg[rofs[i] : rofs[i] + sz], pad=(i > 0))
        xs = xf[e * S : (e + 1) * S]
        for i, sz in enumerate(ssz):
            m[f"xs{i}"] = _pack_x_chunk(xs[sofs[i] : sofs[i] + sz], pad=True)
        in_maps.append(m)

    res = run_bass_kernel_spmd(nc, in_maps, list(range(E)))

    # --- host combine: shared shards + gated scatter-add of routed outputs ---
    out = np.empty((N_TOK, D), dtype=np.float32)
    for e in range(E):
        r = res.results[e]
        out[e * S : (e + 1) * S] = _unpack_out(
            [r[f"os{i}"] for i in range(len(ssz))]
        )

    all_tok = np.concatenate(tok_lists)
    all_contrib = np.concatenate(
        [
            _unpack_out([res.results[e][f"or{i}"] for i in range(len(rsz))])[
                : len(tok_lists[e])
            ]
            * gate_lists[e][:, None].astype(np.float32)
            for e in range(E)
        ]
    )
    pos = np.argsort(all_tok, kind="stable")
    # every token has exactly two routed contributions (top-2 routing)
    out += all_contrib[pos[0::2]]
    out += all_contrib[pos[1::2]]

    return out.reshape(x.shape).astype(np.float32)



# revision 11
# speedup vs baseline: 1.1430x; 1.1430x over previous
"""MoE layer (shared expert + top-2 routed experts) on 8 NeuronCores.

Strategy (expert-parallel, routing-aware):
  - Router (softmax -> top-2 -> renorm) computed on host in float64; it is
    tiny (8192x8) and must match the reference's top-k selection.
  - Core c owns routed expert c: host gathers the tokens routed to expert c
    (~2k of 8192*2 assignments), pads to a common capacity C, and the device
    runs a dense SwiGLU MLP over just those tokens (bf16 matmuls, fp32 accum).
  - The shared expert is data-parallel: core c also runs the shared SwiGLU
    over tokens [c*1024, (c+1)*1024).
  - Combine is done on host: gate-scale each expert's token outputs and
    scatter-add; every token has exactly two routed contributions.

Device layout: activations are kept transposed ([d, tokens]) so the native
[K, M] weight layouts of ew1/ew2/ew3 feed nc.tensor.matmul directly with no
on-device transposes. All matmul inputs are bf16 (PE full rate + FWL),
accumulation is fp32 in PSUM, outputs are written back as bf16.

DMA layout: every dram parameter is packed on the host so that each
dma_start moves one fully-contiguous-per-partition block (4-16 KiB lines).
The baseline's strided layouts produced 1 KiB descriptor fragments, which
made the prologue descriptor-bound: the first real matmul started at
~14.2 us and the PE ran its first ~14 matmuls at the cold 1.2 GHz clock
(HAM re-throttle after a 3.5 us idle gap). Contiguous blocks + a right-
sized PE warmup keep the clock gate open and start the stream ~7 us
earlier. Routed chunks are near-equal (e.g. 5x~428 for C=2136) instead of
512x4+88: N>=128 keeps every chunk at the full weight-load-overlapped
issue rate.
"""

import sys

for _p in ("/opt/trn_rl_repo",):
    if _p not in sys.path:
        sys.path.append(_p)

import numpy as np
import ml_dtypes

import concourse.bass as bass  # noqa: F401  (engine types via nc)
import concourse.mybir as mybir
import concourse.tile as tile
from concourse import bacc
from concourse.bass_utils import run_bass_kernel_spmd

D = 1024
H = 2048
E = 8
N_TOK = 8192  # 4 * 2048
S = N_TOK // E  # shared-expert tokens per core
KD = D // 128  # 8  k-subtiles over d
KH = H // 128  # 16 k-subtiles over h
MH = H // 128  # 16 m-tiles over h
MD = D // 128  # 8  m-tiles over d
NCHUNK = 512
FIRST = 160  # first routed chunk: shorter critical-path DMA to first matmul
LAST = 128  # final shared chunk: short PE->DVE->DMA drain after the last matmul
WARMUP = 28  # PE warmup matmuls issued while the DMA prologue runs

BF = mybir.dt.bfloat16
F32 = mybir.dt.float32

_program_cache: dict[int, tuple] = {}


def _chunk_sizes(total: int, cap: int = NCHUNK) -> list[int]:
    """Split `total` into near-equal chunks of at most `cap` tokens."""
    nch = max(1, -(-total // cap))
    base, rem = divmod(total, nch)
    return [base + (1 if i < rem else 0) for i in range(nch)]


class _WBlocks:
    """Weight tiles split into `ns` column blocks of `bc` columns each."""

    def __init__(self, tiles, ktiles, bc):
        self.tiles = tiles
        self.ktiles = ktiles
        self.bc = bc

    def slice(self, k, m):
        j, c0 = divmod(m * 128, self.bc)
        return self.tiles[j][:, k, c0 : c0 + 128]


def _emit_moe(nc, tc, pools, params):
    """Emit the whole per-core program: routed expert over the gathered
    tokens followed by the shared expert over S tokens. Both are SwiGLU MLPs
    on [d-part, token-free] activations. mm3 of each token-chunk is deferred
    by one chunk so the PE never waits on the DVE h-tile handoff; the first
    chunk of each phase runs all-u-then-all-v so the second weight matrix can
    still be in flight."""
    wpool, xpool, hpool, hspool, opool, ppool, popool = pools
    (w1_ds, w2_ds, w3_ds, s1_ds, s2_ds, s3_ds,
     xg_ds, xs_ds, outr_ds, outs_ds, rsz, ssz) = params

    def load_w(drams, ktiles, tag, defer=False):
        """One dma_start per column block; each block is one contiguous
        per-partition span in DRAM. With defer=True the tiles are created
        but no DMA is emitted — the caller triggers them in an explicit
        priority order via trig()."""
        ts_ = []
        for j, dram in enumerate(drams):
            t = wpool.tile(list(dram.shape), BF, tag=f"{tag}{j}", name=f"{tag}{j}")
            if not defer:
                nc.sync.dma_start(t[:], dram[:])
            ts_.append(t)
        return _WBlocks(ts_, ktiles, drams[0].shape[2])

    def trig(eng, blocks, drams, j):
        eng.dma_start(blocks.tiles[j][:], drams[j][:])

    # chunk-0 tokens first, then w1: the first matmuls need only x0 and the
    # leading w1 block, so those bytes go down the HWDGE FIFO first.
    # x blocks are padded to the full 512-token tile in DRAM so the DMA is
    # contiguous on BOTH sides (8 KiB per-partition spans, not 856 B pieces);
    # chunk 0 gets a dedicated exact-size tile so its critical-path DMA
    # moves only the real tokens. The two k-halves of xc0 and the first two
    # w1 column blocks go down four different engines' HWDGE rings in
    # parallel: the prologue critical path is DMA bandwidth on a single
    # ring, and the first matmul needs all of xc0 plus w1 block 0.
    xc0 = xpool.tile([128, KD, rsz[0]], BF, tag="xc0")
    nc.sync.dma_start(xc0[:, : KD // 2, :], xg_ds[0][:, : KD // 2, :])
    nc.gpsimd.dma_start(xc0[:, KD // 2 :, :], xg_ds[0][:, KD // 2 :, :])
    # PE warmup on an initialized scratch tile while the DMA prologue runs:
    # opens the HAM clock gate so real matmuls start at (or quickly reach)
    # the full 2.4 GHz clock
    warm = wpool.tile([128, 128], BF, tag="warm")
    nc.vector.memset(warm[:], 0.0)
    pwarm = ppool.tile([128, NCHUNK], F32, tag="pu")
    for _ in range(WARMUP):
        nc.tensor.matmul(pwarm[:, :128], warm[:], warm[:], start=True, stop=True)
    # Explicit prologue trigger order. Trigger issue on one engine queue is
    # ~730ns each, so the serial trigger stream itself is a bottleneck: the
    # blocks needed first (w1 m-tiles 0-1 for the u phase, w2 blocks 0-1 for
    # the v phase) go down the otherwise-idle Activation ring while the Sync
    # ring works through the rest in consumption order.
    wa = load_w(w1_ds, KD, "wa", defer=True)
    wb = load_w(w2_ds, KD, "wb", defer=True)
    trig(nc.scalar, wa, w1_ds, 0)
    trig(nc.scalar, wa, w1_ds, 1)
    trig(nc.scalar, wb, w2_ds, 0)
    trig(nc.scalar, wb, w2_ds, 1)
    for j in (4, 5):
        trig(nc.scalar, wa, w1_ds, j)
    for j in (6, 7, 8, 9, 10, 11):
        trig(nc.gpsimd, wa, w1_ds, j)
    for j in (2, 3, 12, 13, 14, 15):
        trig(nc.sync, wa, w1_ds, j)
    for j in (2, 3):
        trig(nc.sync, wb, w2_ds, j)
    xc1 = None
    if len(xg_ds) > 1:
        xc1 = xpool.tile([128, KD, NCHUNK], BF, tag="xc")
        nc.sync.dma_start(xc1[:], xg_ds[1][:])
    # w3 (needed at the first deferred mm3, ~60us in) and shared w1 (needed
    # at the shared phase, ~390us in) are loaded from inside the job loop:
    # the two NeuronCores on one HBM stack otherwise exceed the stack
    # bandwidth during the startup burst, which is what made slow-DMA
    # outlier cores
    wc = sa = sb = sc = None

    def emit_uv(W1, W2, xc, h, nsz, split):
        if split:
            for m in range(MH):
                pu = ppool.tile([128, NCHUNK], F32, tag="pu")
                for k in range(KD):
                    nc.tensor.matmul(
                        pu[:, :nsz],
                        W1.slice(k, m),
                        xc[:, k, :nsz],
                        start=(k == 0),
                        stop=(k == KD - 1),
                    )
                nc.scalar.activation(
                    h[:, m, :nsz], pu[:, :nsz], mybir.ActivationFunctionType.Silu
                )
            for m in range(MH):
                pv = ppool.tile([128, NCHUNK], F32, tag="pv")
                for k in range(KD):
                    nc.tensor.matmul(
                        pv[:, :nsz],
                        W2.slice(k, m),
                        xc[:, k, :nsz],
                        start=(k == 0),
                        stop=(k == KD - 1),
                    )
                nc.vector.tensor_mul(h[:, m, :nsz], h[:, m, :nsz], pv[:, :nsz])
        else:
            for m in range(MH):
                pu = ppool.tile([128, NCHUNK], F32, tag="pu")
                pv = ppool.tile([128, NCHUNK], F32, tag="pv")
                for k in range(KD):
                    nc.tensor.matmul(
                        pu[:, :nsz],
                        W1.slice(k, m),
                        xc[:, k, :nsz],
                        start=(k == 0),
                        stop=(k == KD - 1),
                    )
                for k in range(KD):
                    nc.tensor.matmul(
                        pv[:, :nsz],
                        W2.slice(k, m),
                        xc[:, k, :nsz],
                        start=(k == 0),
                        stop=(k == KD - 1),
                    )
                hs = hspool.tile([128, NCHUNK], F32, tag="hs")
                nc.scalar.activation(
                    hs[:, :nsz], pu[:, :nsz], mybir.ActivationFunctionType.Silu
                )
                nc.vector.tensor_mul(h[:, m, :nsz], hs[:, :nsz], pv[:, :nsz])

    def emit_mm3(W3, h, nsz, out_d, stream):
        ot = opool.tile([128, MD, NCHUNK], BF, tag="ot")
        for mo in range(MD):
            po = popool.tile([128, NCHUNK], F32, tag="po")
            for k in range(KH):
                nc.tensor.matmul(
                    po[:, :nsz],
                    W3.slice(k, mo),
                    h[:, k, :nsz],
                    start=(k == 0),
                    stop=(k == KH - 1),
                )
            nc.vector.tensor_copy(ot[:, mo, :nsz], po[:, :nsz])
            if stream:
                # final chunk: per-mo DMAs so the drain after the last
                # matmul is one small transfer, not the whole chunk
                nc.sync.dma_start(out_d[:, mo, :], ot[:, mo, :nsz])
        if not stream:
            nc.sync.dma_start(out_d[:], ot[:, :, :nsz])

    jobs = [("r", ni) for ni in range(len(xg_ds))]
    jobs += [("s", ni) for ni in range(len(xs_ds))]

    deferred = None
    for ji, (ph, ni) in enumerate(jobs):
        if ji == 1 and wc is None:
            wc = load_w(w3_ds, KH, "wc")
        if (ji == 2 or ph == "s") and sa is None:
            sa = load_w(s1_ds, KD, "sa")  # own slots; prefetch early
        if ph == "r":
            x_d, out_d, W1, W2 = xg_ds[ni], outr_ds[ni], wa, wb
        else:
            if sb is None:
                sb = load_w(s2_ds, KD, "wb")  # reuses w2 slots (WAR-ordered)
            x_d, out_d, W1, W2 = xs_ds[ni], outs_ds[ni], sa, sb
        nsz = (rsz if ph == "r" else ssz)[ni]
        if ji == 0:
            xc = xc0
        elif ji == 1 and xc1 is not None:
            xc = xc1
        else:
            xc = xpool.tile([128, KD, NCHUNK], BF, tag="xc")
            nc.sync.dma_start(xc[:], x_d[:])
        h = hpool.tile([128, KH, NCHUNK], BF, tag="h")
        emit_uv(W1, W2, xc, h, nsz, split=(ni == 0))
        if deferred is not None:
            dph, dh, dnsz, dout = deferred
            emit_mm3(wc if dph == "r" else sc, dh, dnsz, dout, stream=False)
            if ph == "s" and sc is None:
                sc = load_w(s3_ds, KH, "wc")  # reuses w3 slots (WAR-ordered)
        deferred = (ph, h, nsz, out_d)
    dph, dh, dnsz, dout = deferred
    emit_mm3(wc if dph == "r" else sc, dh, dnsz, dout, stream=True)


def _build_program(C: int):
    nc = bacc.Bacc(None, target_bir_lowering=False)

    # small-ish first chunk to start the stream early, near-equal rest;
    # FIRST=320 keeps the chunk-0 u/v phases long enough that the weight
    # FIFO stays ahead even on cores whose DMA starts late
    if C > FIRST + NCHUNK:
        rsz = [FIRST] + _chunk_sizes(C - FIRST)
    else:
        rsz = _chunk_sizes(C)
    # a short LAST chunk keeps the post-stream drain (final PSUM->SBUF copy
    # + output DMA + sem wait) proportional to LAST, not NCHUNK
    ssz = _chunk_sizes(S - LAST) + [LAST]

    def wparams(name, ktiles, ns):
        # w1/w2 free dim is H, w3 free dim is D
        bc = (H if ktiles == KD else D) // ns
        return [
            nc.declare_dram_parameter(f"{name}_{j}", [128, ktiles, bc], BF,
                                      isOutput=False)
            for j in range(ns)
        ]

    w1_ds = wparams("w1", KD, 16)
    w2_ds = wparams("w2", KD, 4)
    w3_ds = wparams("w3", KH, 2)
    s1_ds = wparams("s1", KD, 2)
    s2_ds = wparams("s2", KD, 4)  # must match w2's block shape (slot reuse)
    s3_ds = wparams("s3", KH, 2)  # must match w3's block shape (slot reuse)
    # chunk 0 is exact-size (critical-path DMA); later chunks are padded to
    # the full tile so their DMAs stay contiguous on both sides
    xg_ds = [
        nc.declare_dram_parameter(
            f"xg{i}", [128, KD, sz if i == 0 else NCHUNK], BF, isOutput=False
        )
        for i, sz in enumerate(rsz)
    ]
    xs_ds = [
        nc.declare_dram_parameter(f"xs{i}", [128, KD, NCHUNK], BF, isOutput=False)
        for i, sz in enumerate(ssz)
    ]
    outr_ds = [
        nc.declare_dram_parameter(f"or{i}", [128, MD, sz], BF, isOutput=True)
        for i, sz in enumerate(rsz)
    ]
    outs_ds = [
        nc.declare_dram_parameter(f"os{i}", [128, MD, sz], BF, isOutput=True)
        for i, sz in enumerate(ssz)
    ]

    with tile.TileContext(nc) as tc:
        with (
            tc.tile_pool(name="wpool", bufs=1) as wpool,
            tc.tile_pool(name="xpool", bufs=2) as xpool,
            tc.tile_pool(name="hpool", bufs=2) as hpool,
            tc.tile_pool(name="hspool", bufs=2) as hspool,
            tc.tile_pool(name="opool", bufs=1) as opool,
            tc.tile_pool(name="ppool", bufs=3, space="PSUM") as ppool,
            tc.tile_pool(name="popool", bufs=2, space="PSUM") as popool,
        ):
            pools = (wpool, xpool, hpool, hspool, opool, ppool, popool)
            params = (w1_ds, w2_ds, w3_ds, s1_ds, s2_ds, s3_ds,
                      xg_ds, xs_ds, outr_ds, outs_ds, rsz, ssz)
            _emit_moe(nc, tc, pools, params)

    nc.compile()
    return nc, rsz, ssz


def _get_program(C: int):
    if C not in _program_cache:
        _program_cache[C] = _build_program(C)
    return _program_cache[C]


def _pack_x_chunk(a: np.ndarray, pad: bool) -> np.ndarray:
    """[sz, D] host activation -> [128, KD, sz|NCHUNK] bf16 device block,
    zero-padded to the full tile (except chunk 0) so the DMA is contiguous
    on both sides (d on partitions as d = ko*128 + pi, tokens free)."""
    sz = a.shape[0]
    if pad and sz < NCHUNK:
        a = np.concatenate([a, np.zeros((NCHUNK - sz, D), dtype=a.dtype)])
    b = a.reshape(a.shape[0], KD, 128).transpose(2, 1, 0)
    return np.ascontiguousarray(b.astype(ml_dtypes.bfloat16))


def _pack_w(w: np.ndarray, ns: int) -> list[np.ndarray]:
    """[K, M] host weight (contraction dim first) -> ns blocks of
    [128, K//128, M//ns] bf16, each contiguous per partition."""
    K, M = w.shape
    kk, bc = K // 128, M // ns
    arr = w.reshape(kk, 128, M)
    return [
        np.ascontiguousarray(
            arr[:, :, j * bc : (j + 1) * bc].transpose(1, 0, 2).astype(
                ml_dtypes.bfloat16
            )
        )
        for j in range(ns)
    ]


def _unpack_out(blocks: list[np.ndarray]) -> np.ndarray:
    """Per-chunk [128, MD, sz] bf16 device outputs -> [T, D] fp32 host."""
    outs = []
    for b in blocks:
        sz = b.shape[2]
        outs.append(
            np.asarray(b).astype(np.float32).transpose(2, 1, 0).reshape(sz, D)
        )
    return np.concatenate(outs, axis=0)


def kernel(x, sw1, sw2, sw3, ew1, ew2, ew3, rw, rb):
    x = np.asarray(x, dtype=np.float32)
    sw1, sw2, sw3 = (np.asarray(a, dtype=np.float32) for a in (sw1, sw2, sw3))
    ew1, ew2, ew3 = (np.asarray(a, dtype=np.float32) for a in (ew1, ew2, ew3))
    rw = np.asarray(rw, dtype=np.float32)
    rb = np.asarray(rb, dtype=np.float32)
    xf = np.ascontiguousarray(x.reshape(N_TOK, D), dtype=np.float32)

    # --- host router (float64 to track the fp32 reference's ordering) ---
    logits = xf.astype(np.float64) @ rw.astype(np.float64) + rb.astype(np.float64)
    logits -= logits.max(axis=1, keepdims=True)
    p = np.exp(logits)
    p /= p.sum(axis=1, keepdims=True)
    order = np.argsort(-p, axis=1, kind="stable")
    idx = order[:, :2]  # [N, 2] expert ids, top-2
    w = np.take_along_axis(p, idx, axis=1)
    w = w / w.sum(axis=1, keepdims=True)

    tok_lists = []
    gate_lists = []
    for e in range(E):
        sel = idx == e  # [N, 2]
        any_e = sel.any(axis=1)
        tok = np.nonzero(any_e)[0]
        ge = np.where(sel[tok, 0], w[tok, 0], w[tok, 1])
        tok_lists.append(tok)
        gate_lists.append(ge.astype(np.float64))

    maxT = max(len(t) for t in tok_lists)
    C = max(256, maxT)

    nc, rsz, ssz = _get_program(C)
    rofs = np.cumsum([0] + rsz)
    sofs = np.cumsum([0] + ssz)

    # --- per-core input maps ---
    w1s = [_pack_w(ew1[e], 16) for e in range(E)]
    w2s = [_pack_w(ew2[e], 4) for e in range(E)]
    w3s = [_pack_w(ew3[e], 2) for e in range(E)]
    s1 = _pack_w(sw1, 2)
    s2 = _pack_w(sw2, 4)
    s3 = _pack_w(sw3, 2)

    in_maps = []
    for e in range(E):
        tok = tok_lists[e]
        xg = np.zeros((C, D), dtype=np.float32)
        xg[: len(tok)] = xf[tok]
        m = {}
        for j, blk in enumerate(w1s[e]):
            m[f"w1_{j}"] = blk
        for j, blk in enumerate(w2s[e]):
            m[f"w2_{j}"] = blk
        for j, blk in enumerate(w3s[e]):
            m[f"w3_{j}"] = blk
        for j, blk in enumerate(s1):
            m[f"s1_{j}"] = blk
        for j, blk in enumerate(s2):
            m[f"s2_{j}"] = blk
        for j, blk in enumerate(s3):
            m[f"s3_{j}"] = blk
        for i, sz in enumerate(rsz):
            m[f"xg{i}"] = _pack_x_chunk(xg[rofs[i] : rofs[i] + sz], pad=(i > 0))
        xs = xf[e * S : (e + 1) * S]
        for i, sz in enumerate(ssz):
            m[f"xs{i}"] = _pack_x_chunk(xs[sofs[i] : sofs[i] + sz], pad=True)
        in_maps.append(m)

    res = run_bass_kernel_spmd(nc, in_maps, list(range(E)))

    # --- host combine: shared shards + gated scatter-add of routed outputs ---
    out = np.empty((N_TOK, D), dtype=np.float32)
    for e in range(E):
        r = res.results[e]
        out[e * S : (e + 1) * S] = _unpack_out(
            [r[f"os{i}"] for i in range(len(ssz))]
        )

    all_tok = np.concatenate(tok_lists)
    all_contrib = np.concatenate(
        [
            _unpack_out([res.results[e][f"or{i}"] for i in range(len(rsz))])[
                : len(tok_lists[e])
            ]
            * gate_lists[e][:, None].astype(np.float32)
            for e in range(E)
        ]
    )
    pos = np.argsort(all_tok, kind="stable")
    # every token has exactly two routed contributions (top-2 routing)
    out += all_contrib[pos[0::2]]
    out += all_contrib[pos[1::2]]

    return out.reshape(x.shape).astype(np.float32)



# revision 14
# speedup vs baseline: 1.1652x; 1.0194x over previous
"""MoE layer (shared expert + top-2 routed experts) on 8 NeuronCores.

Strategy (expert-parallel, routing-aware):
  - Router (softmax -> top-2 -> renorm) computed on host in float64; it is
    tiny (8192x8) and must match the reference's top-k selection.
  - Core c owns routed expert c: host gathers the tokens routed to expert c
    (~2k of 8192*2 assignments), pads to a common capacity C, and the device
    runs a dense SwiGLU MLP over just those tokens (bf16 matmuls, fp32 accum).
  - The shared expert is data-parallel: core c also runs the shared SwiGLU
    over tokens [c*1024, (c+1)*1024).
  - Combine is done on host: gate-scale each expert's token outputs and
    scatter-add; every token has exactly two routed contributions.

Device layout: activations are kept transposed ([d, tokens]) so the native
[K, M] weight layouts of ew1/ew2/ew3 feed nc.tensor.matmul directly with no
on-device transposes. All matmul inputs are bf16 (PE full rate + FWL),
accumulation is fp32 in PSUM, outputs are written back as bf16.

DMA layout: every dram parameter is packed on the host so that each
dma_start moves one fully-contiguous-per-partition block (4-16 KiB lines).
The baseline's strided layouts produced 1 KiB descriptor fragments, which
made the prologue descriptor-bound: the first real matmul started at
~14.2 us and the PE ran its first ~14 matmuls at the cold 1.2 GHz clock
(HAM re-throttle after a 3.5 us idle gap). Contiguous blocks + a right-
sized PE warmup keep the clock gate open and start the stream ~7 us
earlier. Routed chunks are near-equal (e.g. 5x~428 for C=2136) instead of
512x4+88: N>=128 keeps every chunk at the full weight-load-overlapped
issue rate.
"""

import sys

for _p in ("/opt/trn_rl_repo",):
    if _p not in sys.path:
        sys.path.append(_p)

import numpy as np
import ml_dtypes

import concourse.bass as bass  # noqa: F401  (engine types via nc)
import concourse.mybir as mybir
import concourse.tile as tile
from concourse import bacc
from concourse.bass_utils import run_bass_kernel_spmd

D = 1024
H = 2048
E = 8
N_TOK = 8192  # 4 * 2048
S = N_TOK // E  # shared-expert tokens per core
KD = D // 128  # 8  k-subtiles over d
KH = H // 128  # 16 k-subtiles over h
MH = H // 128  # 16 m-tiles over h
MD = D // 128  # 8  m-tiles over d
NCHUNK = 512
FIRST = 320  # first routed chunk: enough compute to cover the 8-core DMA burst
LAST = 128  # final shared chunk: short PE->DVE->DMA drain after the last matmul
WARMUP = 44  # PE warmup matmuls issued while the DMA prologue runs

BF = mybir.dt.bfloat16
F32 = mybir.dt.float32

_program_cache: dict[int, tuple] = {}


def _chunk_sizes(total: int, cap: int = NCHUNK) -> list[int]:
    """Split `total` into near-equal chunks of at most `cap` tokens."""
    nch = max(1, -(-total // cap))
    base, rem = divmod(total, nch)
    return [base + (1 if i < rem else 0) for i in range(nch)]


class _WBlocks:
    """Weight tiles split into `ns` column blocks of `bc` columns each."""

    def __init__(self, tiles, ktiles, bc):
        self.tiles = tiles
        self.ktiles = ktiles
        self.bc = bc

    def slice(self, k, m):
        j, c0 = divmod(m * 128, self.bc)
        return self.tiles[j][:, k, c0 : c0 + 128]


def _emit_moe(nc, tc, pools, params):
    """Emit the whole per-core program: routed expert over the gathered
    tokens followed by the shared expert over S tokens. Both are SwiGLU MLPs
    on [d-part, token-free] activations. mm3 of each token-chunk is deferred
    by one chunk so the PE never waits on the DVE h-tile handoff; the first
    chunk of each phase runs all-u-then-all-v so the second weight matrix can
    still be in flight."""
    wpool, xpool, hpool, hspool, opool, ppool, popool = pools
    (w1_ds, w2_ds, w3_ds, s1_ds, s2_ds, s3_ds,
     xg_ds, xs_ds, outr_ds, outs_ds, rsz, ssz) = params

    def load_w(drams, ktiles, tag, defer=False):
        """One dma_start per column block; each block is one contiguous
        per-partition span in DRAM. With defer=True the tiles are created
        but no DMA is emitted — the caller triggers them in an explicit
        priority order via trig()."""
        ts_ = []
        for j, dram in enumerate(drams):
            t = wpool.tile(list(dram.shape), BF, tag=f"{tag}{j}", name=f"{tag}{j}")
            if not defer:
                nc.sync.dma_start(t[:], dram[:])
            ts_.append(t)
        return _WBlocks(ts_, ktiles, drams[0].shape[2])

    def trig(eng, blocks, drams, j):
        eng.dma_start(blocks.tiles[j][:], drams[j][:])

    # chunk-0 tokens first, then w1: the first matmuls need only x0 and the
    # leading w1 block, so those bytes go down the HWDGE FIFO first.
    # x blocks are padded to the full 512-token tile in DRAM so the DMA is
    # contiguous on BOTH sides (8 KiB per-partition spans, not 856 B pieces);
    # chunk 0 gets a dedicated exact-size tile so its critical-path DMA
    # moves only the real tokens. The two k-halves of xc0 and the first two
    # w1 column blocks go down four different engines' HWDGE rings in
    # parallel: the prologue critical path is DMA bandwidth on a single
    # ring, and the first matmul needs all of xc0 plus w1 block 0.
    xc0 = xpool.tile([128, KD, rsz[0]], BF, tag="xc0")
    nc.sync.dma_start(xc0[:, : KD // 2, :], xg_ds[0][:, : KD // 2, :])
    nc.gpsimd.dma_start(xc0[:, KD // 2 :, :], xg_ds[0][:, KD // 2 :, :])
    # PE warmup on an initialized scratch tile while the DMA prologue runs:
    # opens the HAM clock gate so real matmuls start at (or quickly reach)
    # the full 2.4 GHz clock
    warm = wpool.tile([128, 128], BF, tag="warm")
    nc.vector.memset(warm[:], 0.0)
    pwarm = ppool.tile([128, NCHUNK], F32, tag="pu")
    for _ in range(WARMUP):
        nc.tensor.matmul(pwarm[:, :128], warm[:], warm[:], start=True, stop=True)
    # Explicit prologue trigger order. Trigger issue on one engine queue is
    # ~730ns each, so the serial trigger stream itself is a bottleneck: the
    # blocks needed first (w1 m-tiles 0-1 for the u phase, w2 blocks 0-1 for
    # the v phase) go down the otherwise-idle Activation ring while the Sync
    # ring works through the rest in consumption order.
    wa = load_w(w1_ds, KD, "wa", defer=True)
    wb = load_w(w2_ds, KD, "wb", defer=True)
    trig(nc.scalar, wa, w1_ds, 0)
    trig(nc.scalar, wa, w1_ds, 1)
    trig(nc.scalar, wb, w2_ds, 0)
    trig(nc.scalar, wb, w2_ds, 1)
    for j in (4, 5):
        trig(nc.scalar, wa, w1_ds, j)
    for j in (6, 7, 8, 9, 10, 11):
        trig(nc.gpsimd, wa, w1_ds, j)
    for j in (2, 3, 12, 13, 14, 15):
        trig(nc.sync, wa, w1_ds, j)
    for j in (2, 3):
        trig(nc.sync, wb, w2_ds, j)
    xc1 = None
    if len(xg_ds) > 1:
        xc1 = xpool.tile([128, KD, NCHUNK], BF, tag="xc")
        nc.sync.dma_start(xc1[:], xg_ds[1][:])
    # w3 (needed at the first deferred mm3, ~60us in) and shared w1 (needed
    # at the shared phase, ~390us in) are loaded from inside the job loop:
    # the two NeuronCores on one HBM stack otherwise exceed the stack
    # bandwidth during the startup burst, which is what made slow-DMA
    # outlier cores
    wc = sa = sb = sc = None

    def emit_uv(W1, W2, xc, h, nsz, split):
        if split:
            for m in range(MH):
                pu = ppool.tile([128, NCHUNK], F32, tag="pu")
                for k in range(KD):
                    nc.tensor.matmul(
                        pu[:, :nsz],
                        W1.slice(k, m),
                        xc[:, k, :nsz],
                        start=(k == 0),
                        stop=(k == KD - 1),
                    )
                nc.scalar.activation(
                    h[:, m, :nsz], pu[:, :nsz], mybir.ActivationFunctionType.Silu
                )
            for m in range(MH):
                pv = ppool.tile([128, NCHUNK], F32, tag="pv")
                for k in range(KD):
                    nc.tensor.matmul(
                        pv[:, :nsz],
                        W2.slice(k, m),
                        xc[:, k, :nsz],
                        start=(k == 0),
                        stop=(k == KD - 1),
                    )
                nc.vector.tensor_mul(h[:, m, :nsz], h[:, m, :nsz], pv[:, :nsz])
        else:
            for m in range(MH):
                pu = ppool.tile([128, NCHUNK], F32, tag="pu")
                pv = ppool.tile([128, NCHUNK], F32, tag="pv")
                for k in range(KD):
                    nc.tensor.matmul(
                        pu[:, :nsz],
                        W1.slice(k, m),
                        xc[:, k, :nsz],
                        start=(k == 0),
                        stop=(k == KD - 1),
                    )
                for k in range(KD):
                    nc.tensor.matmul(
                        pv[:, :nsz],
                        W2.slice(k, m),
                        xc[:, k, :nsz],
                        start=(k == 0),
                        stop=(k == KD - 1),
                    )
                hs = hspool.tile([128, NCHUNK], F32, tag="hs")
                nc.scalar.activation(
                    hs[:, :nsz], pu[:, :nsz], mybir.ActivationFunctionType.Silu
                )
                nc.vector.tensor_mul(h[:, m, :nsz], hs[:, :nsz], pv[:, :nsz])

    def emit_mm3(W3, h, nsz, out_d, stream):
        ot = opool.tile([128, MD, NCHUNK], BF, tag="ot")
        for mo in range(MD):
            po = popool.tile([128, NCHUNK], F32, tag="po")
            for k in range(KH):
                nc.tensor.matmul(
                    po[:, :nsz],
                    W3.slice(k, mo),
                    h[:, k, :nsz],
                    start=(k == 0),
                    stop=(k == KH - 1),
                )
            nc.vector.tensor_copy(ot[:, mo, :nsz], po[:, :nsz])
            if stream:
                # final chunk: per-mo DMAs so the drain after the last
                # matmul is one small transfer, not the whole chunk
                nc.sync.dma_start(out_d[:, mo, :], ot[:, mo, :nsz])
        if not stream:
            nc.sync.dma_start(out_d[:], ot[:, :, :nsz])

    jobs = [("r", ni) for ni in range(len(xg_ds))]
    jobs += [("s", ni) for ni in range(len(xs_ds))]

    deferred = None
    for ji, (ph, ni) in enumerate(jobs):
        if ji == 1 and wc is None:
            wc = load_w(w3_ds, KH, "wc")
        if (ji == 2 or ph == "s") and sa is None:
            sa = load_w(s1_ds, KD, "sa")  # own slots; prefetch early
        if ph == "r":
            x_d, out_d, W1, W2 = xg_ds[ni], outr_ds[ni], wa, wb
        else:
            if sb is None:
                sb = load_w(s2_ds, KD, "wb")  # reuses w2 slots (WAR-ordered)
            x_d, out_d, W1, W2 = xs_ds[ni], outs_ds[ni], sa, sb
        nsz = (rsz if ph == "r" else ssz)[ni]
        if ji == 0:
            xc = xc0
        elif ji == 1 and xc1 is not None:
            xc = xc1
        else:
            xc = xpool.tile([128, KD, NCHUNK], BF, tag="xc")
            nc.sync.dma_start(xc[:], x_d[:])
        h = hpool.tile([128, KH, NCHUNK], BF, tag="h")
        emit_uv(W1, W2, xc, h, nsz, split=(ni == 0))
        if deferred is not None:
            dph, dh, dnsz, dout = deferred
            emit_mm3(wc if dph == "r" else sc, dh, dnsz, dout, stream=False)
            if ph == "s" and sc is None:
                sc = load_w(s3_ds, KH, "wc")  # reuses w3 slots (WAR-ordered)
        deferred = (ph, h, nsz, out_d)
    dph, dh, dnsz, dout = deferred
    emit_mm3(wc if dph == "r" else sc, dh, dnsz, dout, stream=True)


def _build_program(C: int):
    nc = bacc.Bacc(None, target_bir_lowering=False)

    # small-ish first chunk to start the stream early, near-equal rest;
    # FIRST=320 keeps the chunk-0 u/v phases long enough that the weight
    # FIFO stays ahead even on cores whose DMA starts late
    if C > FIRST + NCHUNK:
        rsz = [FIRST] + _chunk_sizes(C - FIRST)
    else:
        rsz = _chunk_sizes(C)
    # a short LAST chunk keeps the post-stream drain (final PSUM->SBUF copy
    # + output DMA + sem wait) proportional to LAST, not NCHUNK
    ssz = _chunk_sizes(S - LAST) + [LAST]

    def wparams(name, ktiles, ns):
        # w1/w2 free dim is H, w3 free dim is D
        bc = (H if ktiles == KD else D) // ns
        return [
            nc.declare_dram_parameter(f"{name}_{j}", [128, ktiles, bc], BF,
                                      isOutput=False)
            for j in range(ns)
        ]

    w1_ds = wparams("w1", KD, 16)
    w2_ds = wparams("w2", KD, 4)
    w3_ds = wparams("w3", KH, 2)
    s1_ds = wparams("s1", KD, 2)
    s2_ds = wparams("s2", KD, 4)  # must match w2's block shape (slot reuse)
    s3_ds = wparams("s3", KH, 2)  # must match w3's block shape (slot reuse)
    # chunk 0 is exact-size (critical-path DMA); later chunks are padded to
    # the full tile so their DMAs stay contiguous on both sides
    xg_ds = [
        nc.declare_dram_parameter(
            f"xg{i}", [128, KD, sz if i == 0 else NCHUNK], BF, isOutput=False
        )
        for i, sz in enumerate(rsz)
    ]
    xs_ds = [
        nc.declare_dram_parameter(f"xs{i}", [128, KD, NCHUNK], BF, isOutput=False)
        for i, sz in enumerate(ssz)
    ]
    outr_ds = [
        nc.declare_dram_parameter(f"or{i}", [128, MD, sz], BF, isOutput=True)
        for i, sz in enumerate(rsz)
    ]
    outs_ds = [
        nc.declare_dram_parameter(f"os{i}", [128, MD, sz], BF, isOutput=True)
        for i, sz in enumerate(ssz)
    ]

    with tile.TileContext(nc) as tc:
        with (
            tc.tile_pool(name="wpool", bufs=1) as wpool,
            tc.tile_pool(name="xpool", bufs=2) as xpool,
            tc.tile_pool(name="hpool", bufs=2) as hpool,
            tc.tile_pool(name="hspool", bufs=2) as hspool,
            tc.tile_pool(name="opool", bufs=2) as opool,
            tc.tile_pool(name="ppool", bufs=3, space="PSUM") as ppool,
            tc.tile_pool(name="popool", bufs=2, space="PSUM") as popool,
        ):
            pools = (wpool, xpool, hpool, hspool, opool, ppool, popool)
            params = (w1_ds, w2_ds, w3_ds, s1_ds, s2_ds, s3_ds,
                      xg_ds, xs_ds, outr_ds, outs_ds, rsz, ssz)
            _emit_moe(nc, tc, pools, params)

    nc.compile()
    return nc, rsz, ssz


def _get_program(C: int):
    if C not in _program_cache:
        _program_cache[C] = _build_program(C)
    return _program_cache[C]


def _pack_x_chunk(a: np.ndarray, pad: bool) -> np.ndarray:
    """[sz, D] host activation -> [128, KD, sz|NCHUNK] bf16 device block,
    zero-padded to the full tile (except chunk 0) so the DMA is contiguous
    on both sides (d on partitions as d = ko*128 + pi, tokens free)."""
    sz = a.shape[0]
    if pad and sz < NCHUNK:
        a = np.concatenate([a, np.zeros((NCHUNK - sz, D), dtype=a.dtype)])
    b = a.reshape(a.shape[0], KD, 128).transpose(2, 1, 0)
    return np.ascontiguousarray(b.astype(ml_dtypes.bfloat16))


def _pack_w(w: np.ndarray, ns: int) -> list[np.ndarray]:
    """[K, M] host weight (contraction dim first) -> ns blocks of
    [128, K//128, M//ns] bf16, each contiguous per partition."""
    K, M = w.shape
    kk, bc = K // 128, M // ns
    arr = w.reshape(kk, 128, M)
    return [
        np.ascontiguousarray(
            arr[:, :, j * bc : (j + 1) * bc].transpose(1, 0, 2).astype(
                ml_dtypes.bfloat16
            )
        )
        for j in range(ns)
    ]


def _unpack_out(blocks: list[np.ndarray]) -> np.ndarray:
    """Per-chunk [128, MD, sz] bf16 device outputs -> [T, D] fp32 host."""
    outs = []
    for b in blocks:
        sz = b.shape[2]
        outs.append(
            np.asarray(b).astype(np.float32).transpose(2, 1, 0).reshape(sz, D)
        )
    return np.concatenate(outs, axis=0)


def kernel(x, sw1, sw2, sw3, ew1, ew2, ew3, rw, rb):
    x = np.asarray(x, dtype=np.float32)
    sw1, sw2, sw3 = (np.asarray(a, dtype=np.float32) for a in (sw1, sw2, sw3))
    ew1, ew2, ew3 = (np.asarray(a, dtype=np.float32) for a in (ew1, ew2, ew3))
    rw = np.asarray(rw, dtype=np.float32)
    rb = np.asarray(rb, dtype=np.float32)
    xf = np.ascontiguousarray(x.reshape(N_TOK, D), dtype=np.float32)

    # --- host router (float64 to track the fp32 reference's ordering) ---
    logits = xf.astype(np.float64) @ rw.astype(np.float64) + rb.astype(np.float64)
    logits -= logits.max(axis=1, keepdims=True)
    p = np.exp(logits)
    p /= p.sum(axis=1, keepdims=True)
    order = np.argsort(-p, axis=1, kind="stable")
    idx = order[:, :2]  # [N, 2] expert ids, top-2
    w = np.take_along_axis(p, idx, axis=1)
    w = w / w.sum(axis=1, keepdims=True)

    tok_lists = []
    gate_lists = []
    for e in range(E):
        sel = idx == e  # [N, 2]
        any_e = sel.any(axis=1)
        tok = np.nonzero(any_e)[0]
        ge = np.where(sel[tok, 0], w[tok, 0], w[tok, 1])
        tok_lists.append(tok)
        gate_lists.append(ge.astype(np.float64))

    maxT = max(len(t) for t in tok_lists)
    C = max(256, maxT)

    nc, rsz, ssz = _get_program(C)
    rofs = np.cumsum([0] + rsz)
    sofs = np.cumsum([0] + ssz)

    # --- per-core input maps ---
    w1s = [_pack_w(ew1[e], 16) for e in range(E)]
    w2s = [_pack_w(ew2[e], 4) for e in range(E)]
    w3s = [_pack_w(ew3[e], 2) for e in range(E)]
    s1 = _pack_w(sw1, 2)
    s2 = _pack_w(sw2, 4)
    s3 = _pack_w(sw3, 2)

    in_maps = []
    for e in range(E):
        tok = tok_lists[e]
        xg = np.zeros((C, D), dtype=np.float32)
        xg[: len(tok)] = xf[tok]
        m = {}
        for j, blk in enumerate(w1s[e]):
            m[f"w1_{j}"] = blk
        for j, blk in enumerate(w2s[e]):
            m[f"w2_{j}"] = blk
        for j, blk in enumerate(w3s[e]):
            m[f"w3_{j}"] = blk
        for j, blk in enumerate(s1):
            m[f"s1_{j}"] = blk
        for j, blk in enumerate(s2):
            m[f"s2_{j}"] = blk
        for j, blk in enumerate(s3):
            m[f"s3_{j}"] = blk
        for i, sz in enumerate(rsz):
            m[f"xg{i}"] = _pack_x_chunk(xg[rofs[i] : rofs[i] + sz], pad=(i > 0))
        xs = xf[e * S : (e + 1) * S]
        for i, sz in enumerate(ssz):
            m[f"xs{i}"] = _pack_x_chunk(xs[sofs[i] : sofs[i] + sz], pad=True)
        in_maps.append(m)

    res = run_bass_kernel_spmd(nc, in_maps, list(range(E)))

    # --- host combine: shared shards + gated scatter-add of routed outputs ---
    out = np.empty((N_TOK, D), dtype=np.float32)
    for e in range(E):
        r = res.results[e]
        out[e * S : (e + 1) * S] = _unpack_out(
            [r[f"os{i}"] for i in range(len(ssz))]
        )

    all_tok = np.concatenate(tok_lists)
    all_contrib = np.concatenate(
        [
            _unpack_out([res.results[e][f"or{i}"] for i in range(len(rsz))])[
                : len(tok_lists[e])
            ]
            * gate_lists[e][:, None].astype(np.float32)
            for e in range(E)
        ]
    )
    pos = np.argsort(all_tok, kind="stable")
    # every token has exactly two routed contributions (top-2 routing)
    out += all_contrib[pos[0::2]]
    out += all_contrib[pos[1::2]]

    return out.reshape(x.shape).astype(np.float32)



# revision 17
# speedup vs baseline: 1.1688x; 1.0031x over previous
"""MoE layer (shared expert + top-2 routed experts) on 8 NeuronCores.

Strategy (expert-parallel, routing-aware):
  - Router (softmax -> top-2 -> renorm) computed on host in float64; it is
    tiny (8192x8) and must match the reference's top-k selection.
  - Core c owns routed expert c: host gathers the tokens routed to expert c
    (~2k of 8192*2 assignments), pads to a common capacity C, and the device
    runs a dense SwiGLU MLP over just those tokens (bf16 matmuls, fp32 accum).
  - The shared expert is data-parallel: core c also runs the shared SwiGLU
    over tokens [c*1024, (c+1)*1024).
  - Combine is done on host: gate-scale each expert's token outputs and
    scatter-add; every token has exactly two routed contributions.

Device layout: activations are kept transposed ([d, tokens]) so the native
[K, M] weight layouts of ew1/ew2/ew3 feed nc.tensor.matmul directly with no
on-device transposes. All matmul inputs are bf16 (PE full rate + FWL),
accumulation is fp32 in PSUM, outputs are written back as bf16.

DMA layout: every dram parameter is packed on the host so that each
dma_start moves one fully-contiguous-per-partition block (4-16 KiB lines).
The baseline's strided layouts produced 1 KiB descriptor fragments, which
made the prologue descriptor-bound: the first real matmul started at
~14.2 us and the PE ran its first ~14 matmuls at the cold 1.2 GHz clock
(HAM re-throttle after a 3.5 us idle gap). Contiguous blocks + a right-
sized PE warmup keep the clock gate open and start the stream ~7 us
earlier. Routed chunks are near-equal (e.g. 5x~428 for C=2136) instead of
512x4+88: N>=128 keeps every chunk at the full weight-load-overlapped
issue rate.
"""

import sys

for _p in ("/opt/trn_rl_repo",):
    if _p not in sys.path:
        sys.path.append(_p)

import numpy as np
import ml_dtypes

import concourse.bass as bass  # noqa: F401  (engine types via nc)
import concourse.mybir as mybir
import concourse.tile as tile
from concourse import bacc
from concourse.bass_utils import run_bass_kernel_spmd

D = 1024
H = 2048
E = 8
N_TOK = 8192  # 4 * 2048
S = N_TOK // E  # shared-expert tokens per core
KD = D // 128  # 8  k-subtiles over d
KH = H // 128  # 16 k-subtiles over h
MH = H // 128  # 16 m-tiles over h
MD = D // 128  # 8  m-tiles over d
NCHUNK = 512
FIRST = 320  # first routed chunk: enough compute to cover the 8-core DMA burst
LAST = 128  # final shared chunk: short PE->DVE->DMA drain after the last matmul
WARMUP = 24  # PE warmup matmuls issued while the DMA prologue runs

BF = mybir.dt.bfloat16
F32 = mybir.dt.float32

_program_cache: dict[int, tuple] = {}


def _chunk_sizes(total: int, cap: int = NCHUNK) -> list[int]:
    """Split `total` into near-equal chunks of at most `cap` tokens."""
    nch = max(1, -(-total // cap))
    base, rem = divmod(total, nch)
    return [base + (1 if i < rem else 0) for i in range(nch)]


class _WBlocks:
    """Weight tiles split into `ns` column blocks of `bc` columns each."""

    def __init__(self, tiles, ktiles, bc):
        self.tiles = tiles
        self.ktiles = ktiles
        self.bc = bc

    def slice(self, k, m):
        j, c0 = divmod(m * 128, self.bc)
        return self.tiles[j][:, k, c0 : c0 + 128]


def _emit_moe(nc, tc, pools, params):
    """Emit the whole per-core program: routed expert over the gathered
    tokens followed by the shared expert over S tokens. Both are SwiGLU MLPs
    on [d-part, token-free] activations. mm3 of each token-chunk is deferred
    by one chunk so the PE never waits on the DVE h-tile handoff; the first
    chunk of each phase runs all-u-then-all-v so the second weight matrix can
    still be in flight."""
    wpool, xpool, hpool, hspool, opool, ppool, popool = pools
    (w1_ds, w2_ds, w3_ds, s1_ds, s2_ds, s3_ds,
     xg_ds, xs_ds, outr_ds, outs_ds, rsz, ssz) = params

    def load_w(drams, ktiles, tag, defer=False):
        """One dma_start per column block; each block is one contiguous
        per-partition span in DRAM. With defer=True the tiles are created
        but no DMA is emitted — the caller triggers them in an explicit
        priority order via trig()."""
        ts_ = []
        for j, dram in enumerate(drams):
            t = wpool.tile(list(dram.shape), BF, tag=f"{tag}{j}", name=f"{tag}{j}")
            if not defer:
                nc.sync.dma_start(t[:], dram[:])
            ts_.append(t)
        return _WBlocks(ts_, ktiles, drams[0].shape[2])

    def trig(eng, blocks, drams, j):
        eng.dma_start(blocks.tiles[j][:], drams[j][:])

    # chunk-0 tokens first, then w1: the first matmuls need only x0 and the
    # leading w1 block, so those bytes go down the HWDGE FIFO first.
    # x blocks are padded to the full 512-token tile in DRAM so the DMA is
    # contiguous on BOTH sides (8 KiB per-partition spans, not 856 B pieces);
    # chunk 0 gets a dedicated exact-size tile so its critical-path DMA
    # moves only the real tokens. The two k-halves of xc0 and the first two
    # w1 column blocks go down four different engines' HWDGE rings in
    # parallel: the prologue critical path is DMA bandwidth on a single
    # ring, and the first matmul needs all of xc0 plus w1 block 0.
    xc0 = xpool.tile([128, KD, rsz[0]], BF, tag="xc0")
    nc.sync.dma_start(xc0[:, : KD // 2, :], xg_ds[0][:, : KD // 2, :])
    nc.scalar.dma_start(xc0[:, KD // 2 :, :], xg_ds[0][:, KD // 2 :, :])
    # PE warmup on an initialized scratch tile while the DMA prologue runs:
    # opens the HAM clock gate so real matmuls start at (or quickly reach)
    # the full 2.4 GHz clock
    warm = wpool.tile([128, 128], BF, tag="warm")
    nc.vector.memset(warm[:], 0.0)
    pwarm = ppool.tile([128, NCHUNK], F32, tag="pu")
    for _ in range(WARMUP):
        nc.tensor.matmul(pwarm[:, :128], warm[:], warm[:], start=True, stop=True)
    # Explicit prologue trigger order across the TWO HWDGE rings (Sync +
    # Activation). gpsimd DMA is software-DGE and lands blocks ~10us late,
    # so it gets nothing. Each ring receives its blocks in the order the
    # chunk-0 u/v phases consume them; the u-phase m-tiles split ~evenly
    # between the rings so neither has to sustain the full 237 GB/s
    # consumption rate during the 8-core startup burst.
    wa = load_w(w1_ds, KD, "wa", defer=True)
    wb = load_w(w2_ds, KD, "wb", defer=True)
    for j in (0, 1, 4, 5, 6, 7):
        trig(nc.scalar, wa, w1_ds, j)
    for j in (0, 1):
        trig(nc.scalar, wb, w2_ds, j)
    for j in (2, 3, 8, 9, 10, 11, 12, 13, 14, 15):
        trig(nc.sync, wa, w1_ds, j)
    for j in (2, 3):
        trig(nc.sync, wb, w2_ds, j)
    xc1 = None
    if len(xg_ds) > 1:
        xc1 = xpool.tile([128, KD, NCHUNK], BF, tag="xc")
        nc.sync.dma_start(xc1[:], xg_ds[1][:])
    # w3 (needed at the first deferred mm3, ~60us in) and shared w1 (needed
    # at the shared phase, ~390us in) are loaded from inside the job loop:
    # the two NeuronCores on one HBM stack otherwise exceed the stack
    # bandwidth during the startup burst, which is what made slow-DMA
    # outlier cores
    wc = sa = sb = sc = None

    def emit_uv(W1, W2, xc, h, nsz, split):
        if split:
            for m in range(MH):
                pu = ppool.tile([128, NCHUNK], F32, tag="pu")
                for k in range(KD):
                    nc.tensor.matmul(
                        pu[:, :nsz],
                        W1.slice(k, m),
                        xc[:, k, :nsz],
                        start=(k == 0),
                        stop=(k == KD - 1),
                    )
                nc.scalar.activation(
                    h[:, m, :nsz], pu[:, :nsz], mybir.ActivationFunctionType.Silu
                )
            for m in range(MH):
                pv = ppool.tile([128, NCHUNK], F32, tag="pv")
                for k in range(KD):
                    nc.tensor.matmul(
                        pv[:, :nsz],
                        W2.slice(k, m),
                        xc[:, k, :nsz],
                        start=(k == 0),
                        stop=(k == KD - 1),
                    )
                nc.vector.tensor_mul(h[:, m, :nsz], h[:, m, :nsz], pv[:, :nsz])
        else:
            for m in range(MH):
                pu = ppool.tile([128, NCHUNK], F32, tag="pu")
                pv = ppool.tile([128, NCHUNK], F32, tag="pv")
                for k in range(KD):
                    nc.tensor.matmul(
                        pu[:, :nsz],
                        W1.slice(k, m),
                        xc[:, k, :nsz],
                        start=(k == 0),
                        stop=(k == KD - 1),
                    )
                for k in range(KD):
                    nc.tensor.matmul(
                        pv[:, :nsz],
                        W2.slice(k, m),
                        xc[:, k, :nsz],
                        start=(k == 0),
                        stop=(k == KD - 1),
                    )
                hs = hspool.tile([128, NCHUNK], F32, tag="hs")
                nc.scalar.activation(
                    hs[:, :nsz], pu[:, :nsz], mybir.ActivationFunctionType.Silu
                )
                nc.vector.tensor_mul(h[:, m, :nsz], hs[:, :nsz], pv[:, :nsz])

    def emit_mm3(W3, h, nsz, out_d, stream):
        ot = opool.tile([128, MD, NCHUNK], BF, tag="ot")
        for mo in range(MD):
            po = popool.tile([128, NCHUNK], F32, tag="po")
            for k in range(KH):
                nc.tensor.matmul(
                    po[:, :nsz],
                    W3.slice(k, mo),
                    h[:, k, :nsz],
                    start=(k == 0),
                    stop=(k == KH - 1),
                )
            nc.vector.tensor_copy(ot[:, mo, :nsz], po[:, :nsz])
            if stream:
                # final chunk: per-mo DMAs so the drain after the last
                # matmul is one small transfer, not the whole chunk
                nc.sync.dma_start(out_d[:, mo, :], ot[:, mo, :nsz])
        if not stream:
            nc.sync.dma_start(out_d[:], ot[:, :, :nsz])

    jobs = [("r", ni) for ni in range(len(xg_ds))]
    jobs += [("s", ni) for ni in range(len(xs_ds))]

    deferred = None
    for ji, (ph, ni) in enumerate(jobs):
        if ji == 1 and wc is None:
            wc = load_w(w3_ds, KH, "wc")
        if (ji == 2 or ph == "s") and sa is None:
            sa = load_w(s1_ds, KD, "sa")  # own slots; prefetch early
        if ph == "r":
            x_d, out_d, W1, W2 = xg_ds[ni], outr_ds[ni], wa, wb
        else:
            if sb is None:
                sb = load_w(s2_ds, KD, "wb")  # reuses w2 slots (WAR-ordered)
            x_d, out_d, W1, W2 = xs_ds[ni], outs_ds[ni], sa, sb
        nsz = (rsz if ph == "r" else ssz)[ni]
        if ji == 0:
            xc = xc0
        elif ji == 1 and xc1 is not None:
            xc = xc1
        else:
            xc = xpool.tile([128, KD, NCHUNK], BF, tag="xc")
            nc.sync.dma_start(xc[:], x_d[:])
        h = hpool.tile([128, KH, NCHUNK], BF, tag="h")
        emit_uv(W1, W2, xc, h, nsz, split=(ni == 0))
        if deferred is not None:
            dph, dh, dnsz, dout = deferred
            emit_mm3(wc if dph == "r" else sc, dh, dnsz, dout, stream=False)
            if ph == "s" and sc is None:
                sc = load_w(s3_ds, KH, "wc")  # reuses w3 slots (WAR-ordered)
        deferred = (ph, h, nsz, out_d)
    dph, dh, dnsz, dout = deferred
    emit_mm3(wc if dph == "r" else sc, dh, dnsz, dout, stream=True)


def _build_program(C: int):
    nc = bacc.Bacc(None, target_bir_lowering=False)

    # small-ish first chunk to start the stream early, near-equal rest;
    # FIRST=320 keeps the chunk-0 u/v phases long enough that the weight
    # FIFO stays ahead even on cores whose DMA starts late
    if C > FIRST + NCHUNK:
        rsz = [FIRST] + _chunk_sizes(C - FIRST)
    else:
        rsz = _chunk_sizes(C)
    # a short LAST chunk keeps the post-stream drain (final PSUM->SBUF copy
    # + output DMA + sem wait) proportional to LAST, not NCHUNK
    ssz = _chunk_sizes(S - LAST) + [LAST]

    def wparams(name, ktiles, ns):
        # w1/w2 free dim is H, w3 free dim is D
        bc = (H if ktiles == KD else D) // ns
        return [
            nc.declare_dram_parameter(f"{name}_{j}", [128, ktiles, bc], BF,
                                      isOutput=False)
            for j in range(ns)
        ]

    w1_ds = wparams("w1", KD, 16)
    w2_ds = wparams("w2", KD, 4)
    w3_ds = wparams("w3", KH, 2)
    s1_ds = wparams("s1", KD, 2)
    s2_ds = wparams("s2", KD, 4)  # must match w2's block shape (slot reuse)
    s3_ds = wparams("s3", KH, 2)  # must match w3's block shape (slot reuse)
    # chunk 0 is exact-size (critical-path DMA); later chunks are padded to
    # the full tile so their DMAs stay contiguous on both sides
    xg_ds = [
        nc.declare_dram_parameter(
            f"xg{i}", [128, KD, sz if i == 0 else NCHUNK], BF, isOutput=False
        )
        for i, sz in enumerate(rsz)
    ]
    xs_ds = [
        nc.declare_dram_parameter(f"xs{i}", [128, KD, NCHUNK], BF, isOutput=False)
        for i, sz in enumerate(ssz)
    ]
    outr_ds = [
        nc.declare_dram_parameter(f"or{i}", [128, MD, sz], BF, isOutput=True)
        for i, sz in enumerate(rsz)
    ]
    outs_ds = [
        nc.declare_dram_parameter(f"os{i}", [128, MD, sz], BF, isOutput=True)
        for i, sz in enumerate(ssz)
    ]

    with tile.TileContext(nc) as tc:
        with (
            tc.tile_pool(name="wpool", bufs=1) as wpool,
            tc.tile_pool(name="xpool", bufs=2) as xpool,
            tc.tile_pool(name="hpool", bufs=2) as hpool,
            tc.tile_pool(name="hspool", bufs=2) as hspool,
            tc.tile_pool(name="opool", bufs=2) as opool,
            tc.tile_pool(name="ppool", bufs=3, space="PSUM") as ppool,
            tc.tile_pool(name="popool", bufs=2, space="PSUM") as popool,
        ):
            pools = (wpool, xpool, hpool, hspool, opool, ppool, popool)
            params = (w1_ds, w2_ds, w3_ds, s1_ds, s2_ds, s3_ds,
                      xg_ds, xs_ds, outr_ds, outs_ds, rsz, ssz)
            _emit_moe(nc, tc, pools, params)

    nc.compile()
    return nc, rsz, ssz


def _get_program(C: int):
    if C not in _program_cache:
        _program_cache[C] = _build_program(C)
    return _program_cache[C]


def _pack_x_chunk(a: np.ndarray, pad: bool) -> np.ndarray:
    """[sz, D] host activation -> [128, KD, sz|NCHUNK] bf16 device block,
    zero-padded to the full tile (except chunk 0) so the DMA is contiguous
    on both sides (d on partitions as d = ko*128 + pi, tokens free)."""
    sz = a.shape[0]
    if pad and sz < NCHUNK:
        a = np.concatenate([a, np.zeros((NCHUNK - sz, D), dtype=a.dtype)])
    b = a.reshape(a.shape[0], KD, 128).transpose(2, 1, 0)
    return np.ascontiguousarray(b.astype(ml_dtypes.bfloat16))


def _pack_w(w: np.ndarray, ns: int) -> list[np.ndarray]:
    """[K, M] host weight (contraction dim first) -> ns blocks of
    [128, K//128, M//ns] bf16, each contiguous per partition."""
    K, M = w.shape
    kk, bc = K // 128, M // ns
    arr = w.reshape(kk, 128, M)
    return [
        np.ascontiguousarray(
            arr[:, :, j * bc : (j + 1) * bc].transpose(1, 0, 2).astype(
                ml_dtypes.bfloat16
            )
        )
        for j in range(ns)
    ]


def _unpack_out(blocks: list[np.ndarray]) -> np.ndarray:
    """Per-chunk [128, MD, sz] bf16 device outputs -> [T, D] fp32 host."""
    outs = []
    for b in blocks:
        sz = b.shape[2]
        outs.append(
            np.asarray(b).astype(np.float32).transpose(2, 1, 0).reshape(sz, D)
        )
    return np.concatenate(outs, axis=0)


def kernel(x, sw1, sw2, sw3, ew1, ew2, ew3, rw, rb):
    x = np.asarray(x, dtype=np.float32)
    sw1, sw2, sw3 = (np.asarray(a, dtype=np.float32) for a in (sw1, sw2, sw3))
    ew1, ew2, ew3 = (np.asarray(a, dtype=np.float32) for a in (ew1, ew2, ew3))
    rw = np.asarray(rw, dtype=np.float32)
    rb = np.asarray(rb, dtype=np.float32)
    xf = np.ascontiguousarray(x.reshape(N_TOK, D), dtype=np.float32)

    # --- host router (float64 to track the fp32 reference's ordering) ---
    logits = xf.astype(np.float64) @ rw.astype(np.float64) + rb.astype(np.float64)
    logits -= logits.max(axis=1, keepdims=True)
    p = np.exp(logits)
    p /= p.sum(axis=1, keepdims=True)
    order = np.argsort(-p, axis=1, kind="stable")
    idx = order[:, :2]  # [N, 2] expert ids, top-2
    w = np.take_along_axis(p, idx, axis=1)
    w = w / w.sum(axis=1, keepdims=True)

    tok_lists = []
    gate_lists = []
    for e in range(E):
        sel = idx == e  # [N, 2]
        any_e = sel.any(axis=1)
        tok = np.nonzero(any_e)[0]
        ge = np.where(sel[tok, 0], w[tok, 0], w[tok, 1])
        tok_lists.append(tok)
        gate_lists.append(ge.astype(np.float64))

    maxT = max(len(t) for t in tok_lists)
    C = max(256, maxT)

    nc, rsz, ssz = _get_program(C)
    rofs = np.cumsum([0] + rsz)
    sofs = np.cumsum([0] + ssz)

    # --- per-core input maps ---
    w1s = [_pack_w(ew1[e], 16) for e in range(E)]
    w2s = [_pack_w(ew2[e], 4) for e in range(E)]
    w3s = [_pack_w(ew3[e], 2) for e in range(E)]
    s1 = _pack_w(sw1, 2)
    s2 = _pack_w(sw2, 4)
    s3 = _pack_w(sw3, 2)

    in_maps = []
    for e in range(E):
        tok = tok_lists[e]
        xg = np.zeros((C, D), dtype=np.float32)
        xg[: len(tok)] = xf[tok]
        m = {}
        for j, blk in enumerate(w1s[e]):
            m[f"w1_{j}"] = blk
        for j, blk in enumerate(w2s[e]):
            m[f"w2_{j}"] = blk
        for j, blk in enumerate(w3s[e]):
            m[f"w3_{j}"] = blk
        for j, blk in enumerate(s1):
            m[f"s1_{j}"] = blk
        for j, blk in enumerate(s2):
            m[f"s2_{j}"] = blk
        for j, blk in enumerate(s3):
            m[f"s3_{j}"] = blk
        for i, sz in enumerate(rsz):
            m[f"xg{i}"] = _pack_x_chunk(xg[rofs[i] : rofs[i] + sz], pad=(i > 0))
        xs = xf[e * S : (e + 1) * S]
        for i, sz in enumerate(ssz):
            m[f"xs{i}"] = _pack_x_chunk(xs[sofs[i] : sofs[i] + sz], pad=True)
        in_maps.append(m)

    res = run_bass_kernel_spmd(nc, in_maps, list(range(E)))

    # --- host combine: shared shards + gated scatter-add of routed outputs ---
    out = np.empty((N_TOK, D), dtype=np.float32)
    for e in range(E):
        r = res.results[e]
        out[e * S : (e + 1) * S] = _unpack_out(
            [r[f"os{i}"] for i in range(len(ssz))]
        )

    all_tok = np.concatenate(tok_lists)
    all_contrib = np.concatenate(
        [
            _unpack_out([res.results[e][f"or{i}"] for i in range(len(rsz))])[
                : len(tok_lists[e])
            ]
            * gate_lists[e][:, None].astype(np.float32)
            for e in range(E)
        ]
    )
    pos = np.argsort(all_tok, kind="stable")
    # every token has exactly two routed contributions (top-2 routing)
    out += all_contrib[pos[0::2]]
    out += all_contrib[pos[1::2]]

    return out.reshape(x.shape).astype(np.float32)



# revision 20
# speedup vs baseline: 1.1768x; 1.0069x over previous
"""MoE layer (shared expert + top-2 routed experts) on 8 NeuronCores.

Strategy (expert-parallel, routing-aware):
  - Router (softmax -> top-2 -> renorm) computed on host in float64; it is
    tiny (8192x8) and must match the reference's top-k selection.
  - Core c owns routed expert c: host gathers the tokens routed to expert c
    (~2k of 8192*2 assignments), pads to a common capacity C, and the device
    runs a dense SwiGLU MLP over just those tokens (bf16 matmuls, fp32 accum).
  - The shared expert is data-parallel: core c also runs the shared SwiGLU
    over tokens [c*1024, (c+1)*1024).
  - Combine is done on host: gate-scale each expert's token outputs and
    scatter-add; every token has exactly two routed contributions.

Device layout: activations are kept transposed ([d, tokens]) so the native
[K, M] weight layouts of ew1/ew2/ew3 feed nc.tensor.matmul directly with no
on-device transposes. All matmul inputs are bf16 (PE full rate + FWL),
accumulation is fp32 in PSUM, outputs are written back as bf16.

DMA layout: every dram parameter is packed on the host so that each
dma_start moves one fully-contiguous-per-partition block (4-16 KiB lines).
The baseline's strided layouts produced 1 KiB descriptor fragments, which
made the prologue descriptor-bound: the first real matmul started at
~14.2 us and the PE ran its first ~14 matmuls at the cold 1.2 GHz clock
(HAM re-throttle after a 3.5 us idle gap). Contiguous blocks + a right-
sized PE warmup keep the clock gate open and start the stream ~7 us
earlier. Routed chunks are near-equal (e.g. 5x~428 for C=2136) instead of
512x4+88: N>=128 keeps every chunk at the full weight-load-overlapped
issue rate.
"""

import sys

for _p in ("/opt/trn_rl_repo",):
    if _p not in sys.path:
        sys.path.append(_p)

import numpy as np
import ml_dtypes

import concourse.bass as bass  # noqa: F401  (engine types via nc)
import concourse.mybir as mybir
import concourse.tile as tile
from concourse import bacc
from concourse.bass_utils import run_bass_kernel_spmd

D = 1024
H = 2048
E = 8
N_TOK = 8192  # 4 * 2048
S = N_TOK // E  # shared-expert tokens per core
KD = D // 128  # 8  k-subtiles over d
KH = H // 128  # 16 k-subtiles over h
MH = H // 128  # 16 m-tiles over h
MD = D // 128  # 8  m-tiles over d
NCHUNK = 512
FIRST = 320  # first routed chunk: enough compute to cover the 8-core DMA burst
LAST = 128  # final shared chunk: short PE->DVE->DMA drain after the last matmul
WARMUP = 52  # PE warmup matmuls issued while the DMA prologue runs

BF = mybir.dt.bfloat16
F32 = mybir.dt.float32

_program_cache: dict[int, tuple] = {}


def _chunk_sizes(total: int, cap: int = NCHUNK) -> list[int]:
    """Split `total` into near-equal chunks of at most `cap` tokens."""
    nch = max(1, -(-total // cap))
    base, rem = divmod(total, nch)
    return [base + (1 if i < rem else 0) for i in range(nch)]


class _WBlocks:
    """Weight tiles split into `ns` column blocks of `bc` columns each."""

    def __init__(self, tiles, ktiles, bc):
        self.tiles = tiles
        self.ktiles = ktiles
        self.bc = bc

    def slice(self, k, m):
        j, c0 = divmod(m * 128, self.bc)
        return self.tiles[j][:, k, c0 : c0 + 128]


def _emit_moe(nc, tc, pools, params):
    """Emit the whole per-core program: routed expert over the gathered
    tokens followed by the shared expert over S tokens. Both are SwiGLU MLPs
    on [d-part, token-free] activations. mm3 of each token-chunk is deferred
    by one chunk so the PE never waits on the DVE h-tile handoff; the first
    chunk of each phase runs all-u-then-all-v so the second weight matrix can
    still be in flight."""
    wpool, xpool, hpool, hspool, opool, ppool, popool = pools
    (w1_ds, w2_ds, w3_ds, s1_ds, s2_ds, s3_ds,
     xg_ds, xs_ds, outr_ds, outs_ds, rsz, ssz) = params

    def load_w(drams, ktiles, tag, defer=False):
        """One dma_start per column block; each block is one contiguous
        per-partition span in DRAM. With defer=True the tiles are created
        but no DMA is emitted — the caller triggers them in an explicit
        priority order via trig()."""
        ts_ = []
        for j, dram in enumerate(drams):
            t = wpool.tile(list(dram.shape), BF, tag=f"{tag}{j}", name=f"{tag}{j}")
            if not defer:
                nc.sync.dma_start(t[:], dram[:])
            ts_.append(t)
        return _WBlocks(ts_, ktiles, drams[0].shape[2])

    def trig(eng, blocks, drams, j):
        eng.dma_start(blocks.tiles[j][:], drams[j][:])

    # chunk-0 tokens first, then w1: the first matmuls need only x0 and the
    # leading w1 block, so those bytes go down the HWDGE FIFO first.
    # x blocks are padded to the full 512-token tile in DRAM so the DMA is
    # contiguous on BOTH sides (8 KiB per-partition spans, not 856 B pieces);
    # chunk 0 gets a dedicated exact-size tile so its critical-path DMA
    # moves only the real tokens. The two k-halves of xc0 and the first two
    # w1 column blocks go down four different engines' HWDGE rings in
    # parallel: the prologue critical path is DMA bandwidth on a single
    # ring, and the first matmul needs all of xc0 plus w1 block 0.
    xc0 = xpool.tile([128, KD, rsz[0]], BF, tag="xc0")
    nc.sync.dma_start(xc0[:], xg_ds[0][:])
    # PE warmup on an initialized scratch tile while the DMA prologue runs:
    # opens the HAM clock gate so real matmuls start at (or quickly reach)
    # the full 2.4 GHz clock
    warm = wpool.tile([128, 128], BF, tag="warm")
    nc.vector.memset(warm[:], 0.0)
    pwarm = ppool.tile([128, NCHUNK], F32, tag="pu")
    for _ in range(WARMUP):
        nc.tensor.matmul(pwarm[:, :128], warm[:], warm[:], start=True, stop=True)
    # Explicit prologue trigger order across the TWO HWDGE rings. Measured on
    # this machine: the Sync ring sustains ~300-380 GB/s once ramped, the
    # Activation ring only ~100-180 GB/s, and gpsimd DMA is software-DGE
    # (blocks land ~10us late) so it gets nothing. The stream-start critical
    # path (xc0 then w1 blocks 0-9, consumed at one m-tile per 8*FIRST
    # cycles) stays on the fast Sync ring in consumption order; the
    # later-needed blocks (w1 10-15, w2 0-1) ride the slow ring where their
    # deadlines (~26-37us) leave plenty of slack.
    wa = load_w(w1_ds, KD, "wa", defer=True)
    wb = load_w(w2_ds, KD, "wb", defer=True)
    for j in range(0, 10):
        trig(nc.sync, wa, w1_ds, j)
    for j in (10, 11, 12, 13, 14, 15):
        trig(nc.scalar, wa, w1_ds, j)
    for j in (0, 1):
        trig(nc.scalar, wb, w2_ds, j)
    for j in (2, 3):
        trig(nc.sync, wb, w2_ds, j)
    xc1 = None
    if len(xg_ds) > 1:
        xc1 = xpool.tile([128, KD, NCHUNK], BF, tag="xc")
        nc.sync.dma_start(xc1[:], xg_ds[1][:])
    # w3 (needed at the first deferred mm3, ~60us in) and shared w1 (needed
    # at the shared phase, ~390us in) are loaded from inside the job loop:
    # the two NeuronCores on one HBM stack otherwise exceed the stack
    # bandwidth during the startup burst, which is what made slow-DMA
    # outlier cores
    wc = sa = sb = sc = None

    def emit_uv(W1, W2, xc, h, nsz, split):
        if split:
            for m in range(MH):
                pu = ppool.tile([128, NCHUNK], F32, tag="pu")
                for k in range(KD):
                    nc.tensor.matmul(
                        pu[:, :nsz],
                        W1.slice(k, m),
                        xc[:, k, :nsz],
                        start=(k == 0),
                        stop=(k == KD - 1),
                    )
                nc.scalar.activation(
                    h[:, m, :nsz], pu[:, :nsz], mybir.ActivationFunctionType.Silu
                )
            for m in range(MH):
                pv = ppool.tile([128, NCHUNK], F32, tag="pv")
                for k in range(KD):
                    nc.tensor.matmul(
                        pv[:, :nsz],
                        W2.slice(k, m),
                        xc[:, k, :nsz],
                        start=(k == 0),
                        stop=(k == KD - 1),
                    )
                nc.vector.tensor_mul(h[:, m, :nsz], h[:, m, :nsz], pv[:, :nsz])
        else:
            for m in range(MH):
                pu = ppool.tile([128, NCHUNK], F32, tag="pu")
                pv = ppool.tile([128, NCHUNK], F32, tag="pv")
                for k in range(KD):
                    nc.tensor.matmul(
                        pu[:, :nsz],
                        W1.slice(k, m),
                        xc[:, k, :nsz],
                        start=(k == 0),
                        stop=(k == KD - 1),
                    )
                for k in range(KD):
                    nc.tensor.matmul(
                        pv[:, :nsz],
                        W2.slice(k, m),
                        xc[:, k, :nsz],
                        start=(k == 0),
                        stop=(k == KD - 1),
                    )
                hs = hspool.tile([128, NCHUNK], F32, tag="hs")
                nc.scalar.activation(
                    hs[:, :nsz], pu[:, :nsz], mybir.ActivationFunctionType.Silu
                )
                nc.vector.tensor_mul(h[:, m, :nsz], hs[:, :nsz], pv[:, :nsz])

    def emit_mm3(W3, h, nsz, out_d, stream):
        ot = opool.tile([128, MD, NCHUNK], BF, tag="ot")
        for mo in range(MD):
            po = popool.tile([128, NCHUNK], F32, tag="po")
            for k in range(KH):
                nc.tensor.matmul(
                    po[:, :nsz],
                    W3.slice(k, mo),
                    h[:, k, :nsz],
                    start=(k == 0),
                    stop=(k == KH - 1),
                )
            nc.vector.tensor_copy(ot[:, mo, :nsz], po[:, :nsz])
            if stream:
                # final chunk: per-mo DMAs so the drain after the last
                # matmul is one small transfer, not the whole chunk
                nc.sync.dma_start(out_d[:, mo, :], ot[:, mo, :nsz])
        if not stream:
            nc.sync.dma_start(out_d[:], ot[:, :, :nsz])

    jobs = [("r", ni) for ni in range(len(xg_ds))]
    jobs += [("s", ni) for ni in range(len(xs_ds))]

    deferred = None
    for ji, (ph, ni) in enumerate(jobs):
        if ji == 1 and wc is None:
            wc = load_w(w3_ds, KH, "wc")
        if (ji == 2 or ph == "s") and sa is None:
            sa = load_w(s1_ds, KD, "sa")  # own slots; prefetch early
        if ph == "r":
            x_d, out_d, W1, W2 = xg_ds[ni], outr_ds[ni], wa, wb
        else:
            if sb is None:
                sb = load_w(s2_ds, KD, "wb")  # reuses w2 slots (WAR-ordered)
            x_d, out_d, W1, W2 = xs_ds[ni], outs_ds[ni], sa, sb
        nsz = (rsz if ph == "r" else ssz)[ni]
        if ji == 0:
            xc = xc0
        elif ji == 1 and xc1 is not None:
            xc = xc1
        else:
            xc = xpool.tile([128, KD, NCHUNK], BF, tag="xc")
            nc.sync.dma_start(xc[:], x_d[:])
        h = hpool.tile([128, KH, NCHUNK], BF, tag="h")
        emit_uv(W1, W2, xc, h, nsz, split=(ni == 0))
        if deferred is not None:
            dph, dh, dnsz, dout = deferred
            emit_mm3(wc if dph == "r" else sc, dh, dnsz, dout, stream=False)
            if ph == "s" and sc is None:
                sc = load_w(s3_ds, KH, "wc")  # reuses w3 slots (WAR-ordered)
        deferred = (ph, h, nsz, out_d)
    dph, dh, dnsz, dout = deferred
    emit_mm3(wc if dph == "r" else sc, dh, dnsz, dout, stream=True)


def _build_program(C: int):
    nc = bacc.Bacc(None, target_bir_lowering=False)

    # small-ish first chunk to start the stream early, near-equal rest;
    # FIRST=320 keeps the chunk-0 u/v phases long enough that the weight
    # FIFO stays ahead even on cores whose DMA starts late
    if C > FIRST + NCHUNK:
        rsz = [FIRST] + _chunk_sizes(C - FIRST)
    else:
        rsz = _chunk_sizes(C)
    # a short LAST chunk keeps the post-stream drain (final PSUM->SBUF copy
    # + output DMA + sem wait) proportional to LAST, not NCHUNK
    ssz = _chunk_sizes(S - LAST) + [LAST]

    def wparams(name, ktiles, ns):
        # w1/w2 free dim is H, w3 free dim is D
        bc = (H if ktiles == KD else D) // ns
        return [
            nc.declare_dram_parameter(f"{name}_{j}", [128, ktiles, bc], BF,
                                      isOutput=False)
            for j in range(ns)
        ]

    w1_ds = wparams("w1", KD, 16)
    w2_ds = wparams("w2", KD, 4)
    w3_ds = wparams("w3", KH, 2)
    s1_ds = wparams("s1", KD, 2)
    s2_ds = wparams("s2", KD, 4)  # must match w2's block shape (slot reuse)
    s3_ds = wparams("s3", KH, 2)  # must match w3's block shape (slot reuse)
    # chunk 0 is exact-size (critical-path DMA); later chunks are padded to
    # the full tile so their DMAs stay contiguous on both sides
    xg_ds = [
        nc.declare_dram_parameter(
            f"xg{i}", [128, KD, sz if i == 0 else NCHUNK], BF, isOutput=False
        )
        for i, sz in enumerate(rsz)
    ]
    xs_ds = [
        nc.declare_dram_parameter(f"xs{i}", [128, KD, NCHUNK], BF, isOutput=False)
        for i, sz in enumerate(ssz)
    ]
    outr_ds = [
        nc.declare_dram_parameter(f"or{i}", [128, MD, sz], BF, isOutput=True)
        for i, sz in enumerate(rsz)
    ]
    outs_ds = [
        nc.declare_dram_parameter(f"os{i}", [128, MD, sz], BF, isOutput=True)
        for i, sz in enumerate(ssz)
    ]

    with tile.TileContext(nc) as tc:
        with (
            tc.tile_pool(name="wpool", bufs=1) as wpool,
            tc.tile_pool(name="xpool", bufs=2) as xpool,
            tc.tile_pool(name="hpool", bufs=2) as hpool,
            tc.tile_pool(name="hspool", bufs=2) as hspool,
            tc.tile_pool(name="opool", bufs=2) as opool,
            tc.tile_pool(name="ppool", bufs=3, space="PSUM") as ppool,
            tc.tile_pool(name="popool", bufs=2, space="PSUM") as popool,
        ):
            pools = (wpool, xpool, hpool, hspool, opool, ppool, popool)
            params = (w1_ds, w2_ds, w3_ds, s1_ds, s2_ds, s3_ds,
                      xg_ds, xs_ds, outr_ds, outs_ds, rsz, ssz)
            _emit_moe(nc, tc, pools, params)

    nc.compile()
    return nc, rsz, ssz


def _get_program(C: int):
    if C not in _program_cache:
        _program_cache[C] = _build_program(C)
    return _program_cache[C]


def _pack_x_chunk(a: np.ndarray, pad: bool) -> np.ndarray:
    """[sz, D] host activation -> [128, KD, sz|NCHUNK] bf16 device block,
    zero-padded to the full tile (except chunk 0) so the DMA is contiguous
    on both sides (d on partitions as d = ko*128 + pi, tokens free)."""
    sz = a.shape[0]
    if pad and sz < NCHUNK:
        a = np.concatenate([a, np.zeros((NCHUNK - sz, D), dtype=a.dtype)])
    b = a.reshape(a.shape[0], KD, 128).transpose(2, 1, 0)
    return np.ascontiguousarray(b.astype(ml_dtypes.bfloat16))


def _pack_w(w: np.ndarray, ns: int) -> list[np.ndarray]:
    """[K, M] host weight (contraction dim first) -> ns blocks of
    [128, K//128, M//ns] bf16, each contiguous per partition."""
    K, M = w.shape
    kk, bc = K // 128, M // ns
    arr = w.reshape(kk, 128, M)
    return [
        np.ascontiguousarray(
            arr[:, :, j * bc : (j + 1) * bc].transpose(1, 0, 2).astype(
                ml_dtypes.bfloat16
            )
        )
        for j in range(ns)
    ]


def _unpack_out(blocks: list[np.ndarray]) -> np.ndarray:
    """Per-chunk [128, MD, sz] bf16 device outputs -> [T, D] fp32 host."""
    outs = []
    for b in blocks:
        sz = b.shape[2]
        outs.append(
            np.asarray(b).astype(np.float32).transpose(2, 1, 0).reshape(sz, D)
        )
    return np.concatenate(outs, axis=0)


def kernel(x, sw1, sw2, sw3, ew1, ew2, ew3, rw, rb):
    x = np.asarray(x, dtype=np.float32)
    sw1, sw2, sw3 = (np.asarray(a, dtype=np.float32) for a in (sw1, sw2, sw3))
    ew1, ew2, ew3 = (np.asarray(a, dtype=np.float32) for a in (ew1, ew2, ew3))
    rw = np.asarray(rw, dtype=np.float32)
    rb = np.asarray(rb, dtype=np.float32)
    xf = np.ascontiguousarray(x.reshape(N_TOK, D), dtype=np.float32)

    # --- host router (float64 to track the fp32 reference's ordering) ---
    logits = xf.astype(np.float64) @ rw.astype(np.float64) + rb.astype(np.float64)
    logits -= logits.max(axis=1, keepdims=True)
    p = np.exp(logits)
    p /= p.sum(axis=1, keepdims=True)
    order = np.argsort(-p, axis=1, kind="stable")
    idx = order[:, :2]  # [N, 2] expert ids, top-2
    w = np.take_along_axis(p, idx, axis=1)
    w = w / w.sum(axis=1, keepdims=True)

    tok_lists = []
    gate_lists = []
    for e in range(E):
        sel = idx == e  # [N, 2]
        any_e = sel.any(axis=1)
        tok = np.nonzero(any_e)[0]
        ge = np.where(sel[tok, 0], w[tok, 0], w[tok, 1])
        tok_lists.append(tok)
        gate_lists.append(ge.astype(np.float64))

    maxT = max(len(t) for t in tok_lists)
    C = max(256, maxT)

    nc, rsz, ssz = _get_program(C)
    rofs = np.cumsum([0] + rsz)
    sofs = np.cumsum([0] + ssz)

    # --- per-core input maps ---
    w1s = [_pack_w(ew1[e], 16) for e in range(E)]
    w2s = [_pack_w(ew2[e], 4) for e in range(E)]
    w3s = [_pack_w(ew3[e], 2) for e in range(E)]
    s1 = _pack_w(sw1, 2)
    s2 = _pack_w(sw2, 4)
    s3 = _pack_w(sw3, 2)

    in_maps = []
    for e in range(E):
        tok = tok_lists[e]
        xg = np.zeros((C, D), dtype=np.float32)
        xg[: len(tok)] = xf[tok]
        m = {}
        for j, blk in enumerate(w1s[e]):
            m[f"w1_{j}"] = blk
        for j, blk in enumerate(w2s[e]):
            m[f"w2_{j}"] = blk
        for j, blk in enumerate(w3s[e]):
            m[f"w3_{j}"] = blk
        for j, blk in enumerate(s1):
            m[f"s1_{j}"] = blk
        for j, blk in enumerate(s2):
            m[f"s2_{j}"] = blk
        for j, blk in enumerate(s3):
            m[f"s3_{j}"] = blk
        for i, sz in enumerate(rsz):
            m[f"xg{i}"] = _pack_x_chunk(xg[rofs[i] : rofs[i] + sz], pad=(i > 0))
        xs = xf[e * S : (e + 1) * S]
        for i, sz in enumerate(ssz):
            m[f"xs{i}"] = _pack_x_chunk(xs[sofs[i] : sofs[i] + sz], pad=True)
        in_maps.append(m)

    res = run_bass_kernel_spmd(nc, in_maps, list(range(E)))

    # --- host combine: shared shards + gated scatter-add of routed outputs ---
    out = np.empty((N_TOK, D), dtype=np.float32)
    for e in range(E):
        r = res.results[e]
        out[e * S : (e + 1) * S] = _unpack_out(
            [r[f"os{i}"] for i in range(len(ssz))]
        )

    all_tok = np.concatenate(tok_lists)
    all_contrib = np.concatenate(
        [
            _unpack_out([res.results[e][f"or{i}"] for i in range(len(rsz))])[
                : len(tok_lists[e])
            ]
            * gate_lists[e][:, None].astype(np.float32)
            for e in range(E)
        ]
    )
    pos = np.argsort(all_tok, kind="stable")
    # every token has exactly two routed contributions (top-2 routing)
    out += all_contrib[pos[0::2]]
    out += all_contrib[pos[1::2]]

    return out.reshape(x.shape).astype(np.float32)



# revision 22
# speedup vs baseline: 1.1991x; 1.0189x over previous
"""MoE layer (shared expert + top-2 routed experts) on 8 NeuronCores.

Strategy (expert-parallel, routing-aware):
  - Router (softmax -> top-2 -> renorm) computed on host in float64; it is
    tiny (8192x8) and must match the reference's top-k selection.
  - Core c owns routed expert c: host gathers the tokens routed to expert c
    (~2k of 8192*2 assignments), pads to a common capacity C, and the device
    runs a dense SwiGLU MLP over just those tokens (bf16 matmuls, fp32 accum).
  - The shared expert is data-parallel: core c also runs the shared SwiGLU
    over tokens [c*1024, (c+1)*1024).
  - Combine is done on host: gate-scale each expert's token outputs and
    scatter-add; every token has exactly two routed contributions.

Device layout: activations are kept transposed ([d, tokens]) so the native
[K, M] weight layouts of ew1/ew2/ew3 feed nc.tensor.matmul directly with no
on-device transposes. All matmul inputs are bf16 (PE full rate + FWL),
accumulation is fp32 in PSUM, outputs are written back as bf16.

DMA layout: every dram parameter is packed on the host so that each
dma_start moves one fully-contiguous-per-partition block (4-16 KiB lines).
The baseline's strided layouts produced 1 KiB descriptor fragments, which
made the prologue descriptor-bound: the first real matmul started at
~14.2 us and the PE ran its first ~14 matmuls at the cold 1.2 GHz clock
(HAM re-throttle after a 3.5 us idle gap). Contiguous blocks + a right-
sized PE warmup keep the clock gate open and start the stream ~7 us
earlier. Routed chunks are near-equal (e.g. 5x~428 for C=2136) instead of
512x4+88: N>=128 keeps every chunk at the full weight-load-overlapped
issue rate.
"""

import sys

for _p in ("/opt/trn_rl_repo",):
    if _p not in sys.path:
        sys.path.append(_p)

import numpy as np
import ml_dtypes

import concourse.bass as bass  # noqa: F401  (engine types via nc)
import concourse.mybir as mybir
import concourse.tile as tile
from concourse import bacc
from concourse.bass_utils import run_bass_kernel_spmd

D = 1024
H = 2048
E = 8
N_TOK = 8192  # 4 * 2048
S = N_TOK // E  # shared-expert tokens per core
KD = D // 128  # 8  k-subtiles over d
KH = H // 128  # 16 k-subtiles over h
MH = H // 128  # 16 m-tiles over h
MD = D // 128  # 8  m-tiles over d
NCHUNK = 512
FIRST = 320  # first routed chunk: enough compute to cover the 8-core DMA burst
LAST = 128  # final shared chunk: short PE->DVE->DMA drain after the last matmul
WARMUP = 56  # PE warmup matmuls issued while the DMA prologue runs

BF = mybir.dt.bfloat16
F32 = mybir.dt.float32

_program_cache: dict[int, tuple] = {}


def _chunk_sizes(total: int, cap: int = NCHUNK) -> list[int]:
    """Split `total` into near-equal chunks of at most `cap` tokens."""
    nch = max(1, -(-total // cap))
    base, rem = divmod(total, nch)
    return [base + (1 if i < rem else 0) for i in range(nch)]


class _WBlocks:
    """Weight tiles split into `ns` column blocks of `bc` columns each."""

    def __init__(self, tiles, ktiles, bc):
        self.tiles = tiles
        self.ktiles = ktiles
        self.bc = bc

    def slice(self, k, m):
        j, c0 = divmod(m * 128, self.bc)
        return self.tiles[j][:, k, c0 : c0 + 128]


def _emit_moe(nc, tc, pools, params):
    """Emit the whole per-core program: routed expert over the gathered
    tokens followed by the shared expert over S tokens. Both are SwiGLU MLPs
    on [d-part, token-free] activations. mm3 of each token-chunk is deferred
    by one chunk so the PE never waits on the DVE h-tile handoff; the first
    chunk of each phase runs all-u-then-all-v so the second weight matrix can
    still be in flight."""
    wpool, xpool, hpool, hspool, opool, ppool, popool = pools
    (w1_ds, w2_ds, w3_ds, s1_ds, s2_ds, s3_ds,
     xg_ds, xs_ds, outr_ds, outs_ds, rsz, ssz) = params

    def load_w(drams, ktiles, tag, defer=False):
        """One dma_start per column block; each block is one contiguous
        per-partition span in DRAM. With defer=True the tiles are created
        but no DMA is emitted — the caller triggers them in an explicit
        priority order via trig()."""
        ts_ = []
        for j, dram in enumerate(drams):
            t = wpool.tile(list(dram.shape), BF, tag=f"{tag}{j}", name=f"{tag}{j}")
            if not defer:
                nc.sync.dma_start(t[:], dram[:])
            ts_.append(t)
        return _WBlocks(ts_, ktiles, drams[0].shape[2])

    def trig(eng, blocks, drams, j):
        eng.dma_start(blocks.tiles[j][:], drams[j][:])

    # chunk-0 tokens first, then w1: the first matmuls need only x0 and the
    # leading w1 block, so those bytes go down the HWDGE FIFO first.
    # x blocks are padded to the full 512-token tile in DRAM so the DMA is
    # contiguous on BOTH sides (8 KiB per-partition spans, not 856 B pieces);
    # chunk 0 gets a dedicated exact-size tile so its critical-path DMA
    # moves only the real tokens. The two k-halves of xc0 and the first two
    # w1 column blocks go down four different engines' HWDGE rings in
    # parallel: the prologue critical path is DMA bandwidth on a single
    # ring, and the first matmul needs all of xc0 plus w1 block 0.
    xc0 = xpool.tile([128, KD, rsz[0]], BF, tag="xc0")
    nc.sync.dma_start(xc0[:], xg_ds[0][:])
    # PE warmup on an initialized scratch tile while the DMA prologue runs:
    # opens the HAM clock gate so real matmuls start at (or quickly reach)
    # the full 2.4 GHz clock
    warm = wpool.tile([128, 128], BF, tag="warm")
    nc.vector.memset(warm[:], 0.0)
    pwarm = ppool.tile([128, NCHUNK], F32, tag="pu")
    for _ in range(WARMUP):
        nc.tensor.matmul(pwarm[:, :128], warm[:], warm[:], start=True, stop=True)
    # All prologue weight blocks ride the single Sync HWDGE ring in exact
    # consumption order (xc0, w1 blocks 0..15, w2 blocks 0..3). Measured on
    # this machine: the early 8-core startup burst caps each ring at
    # ~100-180 GB/s, so offloading the later blocks to the (2x slower)
    # Activation ring just steals HBM bandwidth from the urgent blocks and
    # moves the stall around; a single ring drained in deadline order is
    # self-pacing. gpsimd DMA is software-DGE (lands ~10us late) - unusable.
    wa = load_w(w1_ds, KD, "wa")
    wb = load_w(w2_ds, KD, "wb")
    xc1 = None
    if len(xg_ds) > 1:
        xc1 = xpool.tile([128, KD, NCHUNK], BF, tag="xc")
        nc.sync.dma_start(xc1[:], xg_ds[1][:])
    # w3 (needed at the first deferred mm3, ~60us in) and shared w1 (needed
    # at the shared phase, ~390us in) are loaded from inside the job loop:
    # the two NeuronCores on one HBM stack otherwise exceed the stack
    # bandwidth during the startup burst, which is what made slow-DMA
    # outlier cores
    wc = sa = sb = sc = None

    def emit_uv(W1, W2, xc, h, nsz, split):
        if split:
            for m in range(MH):
                pu = ppool.tile([128, NCHUNK], F32, tag="pu")
                for k in range(KD):
                    nc.tensor.matmul(
                        pu[:, :nsz],
                        W1.slice(k, m),
                        xc[:, k, :nsz],
                        start=(k == 0),
                        stop=(k == KD - 1),
                    )
                nc.scalar.activation(
                    h[:, m, :nsz], pu[:, :nsz], mybir.ActivationFunctionType.Silu
                )
            for m in range(MH):
                pv = ppool.tile([128, NCHUNK], F32, tag="pv")
                for k in range(KD):
                    nc.tensor.matmul(
                        pv[:, :nsz],
                        W2.slice(k, m),
                        xc[:, k, :nsz],
                        start=(k == 0),
                        stop=(k == KD - 1),
                    )
                nc.vector.tensor_mul(h[:, m, :nsz], h[:, m, :nsz], pv[:, :nsz])
        else:
            for m in range(MH):
                pu = ppool.tile([128, NCHUNK], F32, tag="pu")
                pv = ppool.tile([128, NCHUNK], F32, tag="pv")
                for k in range(KD):
                    nc.tensor.matmul(
                        pu[:, :nsz],
                        W1.slice(k, m),
                        xc[:, k, :nsz],
                        start=(k == 0),
                        stop=(k == KD - 1),
                    )
                for k in range(KD):
                    nc.tensor.matmul(
                        pv[:, :nsz],
                        W2.slice(k, m),
                        xc[:, k, :nsz],
                        start=(k == 0),
                        stop=(k == KD - 1),
                    )
                hs = hspool.tile([128, NCHUNK], F32, tag="hs")
                nc.scalar.activation(
                    hs[:, :nsz], pu[:, :nsz], mybir.ActivationFunctionType.Silu
                )
                nc.vector.tensor_mul(h[:, m, :nsz], hs[:, :nsz], pv[:, :nsz])

    def emit_mm3(W3, h, nsz, out_d, stream):
        ot = opool.tile([128, MD, NCHUNK], BF, tag="ot")
        for mo in range(MD):
            po = popool.tile([128, NCHUNK], F32, tag="po")
            for k in range(KH):
                nc.tensor.matmul(
                    po[:, :nsz],
                    W3.slice(k, mo),
                    h[:, k, :nsz],
                    start=(k == 0),
                    stop=(k == KH - 1),
                )
            nc.vector.tensor_copy(ot[:, mo, :nsz], po[:, :nsz])
            if stream:
                # final chunk: per-mo DMAs so the drain after the last
                # matmul is one small transfer, not the whole chunk
                nc.sync.dma_start(out_d[:, mo, :], ot[:, mo, :nsz])
        if not stream:
            nc.sync.dma_start(out_d[:], ot[:, :, :nsz])

    jobs = [("r", ni) for ni in range(len(xg_ds))]
    jobs += [("s", ni) for ni in range(len(xs_ds))]

    deferred = None
    for ji, (ph, ni) in enumerate(jobs):
        if ji == 1 and wc is None:
            wc = load_w(w3_ds, KH, "wc")
        if (ji == 2 or ph == "s") and sa is None:
            sa = load_w(s1_ds, KD, "sa")  # own slots; prefetch early
        if ph == "r":
            x_d, out_d, W1, W2 = xg_ds[ni], outr_ds[ni], wa, wb
        else:
            if sb is None:
                sb = load_w(s2_ds, KD, "wb")  # reuses w2 slots (WAR-ordered)
            x_d, out_d, W1, W2 = xs_ds[ni], outs_ds[ni], sa, sb
        nsz = (rsz if ph == "r" else ssz)[ni]
        if ji == 0:
            xc = xc0
        elif ji == 1 and xc1 is not None:
            xc = xc1
        else:
            xc = xpool.tile([128, KD, NCHUNK], BF, tag="xc")
            nc.sync.dma_start(xc[:], x_d[:])
        h = hpool.tile([128, KH, NCHUNK], BF, tag="h")
        emit_uv(W1, W2, xc, h, nsz, split=(ni == 0))
        if deferred is not None:
            dph, dh, dnsz, dout = deferred
            emit_mm3(wc if dph == "r" else sc, dh, dnsz, dout, stream=False)
            if ph == "s" and sc is None:
                sc = load_w(s3_ds, KH, "wc")  # reuses w3 slots (WAR-ordered)
        deferred = (ph, h, nsz, out_d)
    dph, dh, dnsz, dout = deferred
    emit_mm3(wc if dph == "r" else sc, dh, dnsz, dout, stream=True)


def _build_program(C: int):
    nc = bacc.Bacc(None, target_bir_lowering=False)

    # small-ish first chunk to start the stream early, near-equal rest;
    # FIRST=320 keeps the chunk-0 u/v phases long enough that the weight
    # FIFO stays ahead even on cores whose DMA starts late
    if C > FIRST + NCHUNK:
        rsz = [FIRST] + _chunk_sizes(C - FIRST)
    else:
        rsz = _chunk_sizes(C)
    # a short LAST chunk keeps the post-stream drain (final PSUM->SBUF copy
    # + output DMA + sem wait) proportional to LAST, not NCHUNK
    ssz = _chunk_sizes(S - LAST) + [LAST]

    def wparams(name, ktiles, ns):
        # w1/w2 free dim is H, w3 free dim is D
        bc = (H if ktiles == KD else D) // ns
        return [
            nc.declare_dram_parameter(f"{name}_{j}", [128, ktiles, bc], BF,
                                      isOutput=False)
            for j in range(ns)
        ]

    w1_ds = wparams("w1", KD, 16)
    w2_ds = wparams("w2", KD, 4)
    w3_ds = wparams("w3", KH, 2)
    s1_ds = wparams("s1", KD, 2)
    s2_ds = wparams("s2", KD, 4)  # must match w2's block shape (slot reuse)
    s3_ds = wparams("s3", KH, 2)  # must match w3's block shape (slot reuse)
    # chunk 0 is exact-size (critical-path DMA); later chunks are padded to
    # the full tile so their DMAs stay contiguous on both sides
    xg_ds = [
        nc.declare_dram_parameter(
            f"xg{i}", [128, KD, sz if i == 0 else NCHUNK], BF, isOutput=False
        )
        for i, sz in enumerate(rsz)
    ]
    xs_ds = [
        nc.declare_dram_parameter(f"xs{i}", [128, KD, NCHUNK], BF, isOutput=False)
        for i, sz in enumerate(ssz)
    ]
    outr_ds = [
        nc.declare_dram_parameter(f"or{i}", [128, MD, sz], BF, isOutput=True)
        for i, sz in enumerate(rsz)
    ]
    outs_ds = [
        nc.declare_dram_parameter(f"os{i}", [128, MD, sz], BF, isOutput=True)
        for i, sz in enumerate(ssz)
    ]

    with tile.TileContext(nc) as tc:
        with (
            tc.tile_pool(name="wpool", bufs=1) as wpool,
            tc.tile_pool(name="xpool", bufs=2) as xpool,
            tc.tile_pool(name="hpool", bufs=2) as hpool,
            tc.tile_pool(name="hspool", bufs=2) as hspool,
            tc.tile_pool(name="opool", bufs=2) as opool,
            tc.tile_pool(name="ppool", bufs=3, space="PSUM") as ppool,
            tc.tile_pool(name="popool", bufs=2, space="PSUM") as popool,
        ):
            pools = (wpool, xpool, hpool, hspool, opool, ppool, popool)
            params = (w1_ds, w2_ds, w3_ds, s1_ds, s2_ds, s3_ds,
                      xg_ds, xs_ds, outr_ds, outs_ds, rsz, ssz)
            _emit_moe(nc, tc, pools, params)

    nc.compile()
    return nc, rsz, ssz


def _get_program(C: int):
    if C not in _program_cache:
        _program_cache[C] = _build_program(C)
    return _program_cache[C]


def _pack_x_chunk(a: np.ndarray, pad: bool) -> np.ndarray:
    """[sz, D] host activation -> [128, KD, sz|NCHUNK] bf16 device block,
    zero-padded to the full tile (except chunk 0) so the DMA is contiguous
    on both sides (d on partitions as d = ko*128 + pi, tokens free)."""
    sz = a.shape[0]
    if pad and sz < NCHUNK:
        a = np.concatenate([a, np.zeros((NCHUNK - sz, D), dtype=a.dtype)])
    b = a.reshape(a.shape[0], KD, 128).transpose(2, 1, 0)
    return np.ascontiguousarray(b.astype(ml_dtypes.bfloat16))


def _pack_w(w: np.ndarray, ns: int) -> list[np.ndarray]:
    """[K, M] host weight (contraction dim first) -> ns blocks of
    [128, K//128, M//ns] bf16, each contiguous per partition."""
    K, M = w.shape
    kk, bc = K // 128, M // ns
    arr = w.reshape(kk, 128, M)
    return [
        np.ascontiguousarray(
            arr[:, :, j * bc : (j + 1) * bc].transpose(1, 0, 2).astype(
                ml_dtypes.bfloat16
            )
        )
        for j in range(ns)
    ]


def _unpack_out(blocks: list[np.ndarray]) -> np.ndarray:
    """Per-chunk [128, MD, sz] bf16 device outputs -> [T, D] fp32 host."""
    outs = []
    for b in blocks:
        sz = b.shape[2]
        outs.append(
            np.asarray(b).astype(np.float32).transpose(2, 1, 0).reshape(sz, D)
        )
    return np.concatenate(outs, axis=0)


def kernel(x, sw1, sw2, sw3, ew1, ew2, ew3, rw, rb):
    x = np.asarray(x, dtype=np.float32)
    sw1, sw2, sw3 = (np.asarray(a, dtype=np.float32) for a in (sw1, sw2, sw3))
    ew1, ew2, ew3 = (np.asarray(a, dtype=np.float32) for a in (ew1, ew2, ew3))
    rw = np.asarray(rw, dtype=np.float32)
    rb = np.asarray(rb, dtype=np.float32)
    xf = np.ascontiguousarray(x.reshape(N_TOK, D), dtype=np.float32)

    # --- host router (float64 to track the fp32 reference's ordering) ---
    logits = xf.astype(np.float64) @ rw.astype(np.float64) + rb.astype(np.float64)
    logits -= logits.max(axis=1, keepdims=True)
    p = np.exp(logits)
    p /= p.sum(axis=1, keepdims=True)
    order = np.argsort(-p, axis=1, kind="stable")
    idx = order[:, :2]  # [N, 2] expert ids, top-2
    w = np.take_along_axis(p, idx, axis=1)
    w = w / w.sum(axis=1, keepdims=True)

    tok_lists = []
    gate_lists = []
    for e in range(E):
        sel = idx == e  # [N, 2]
        any_e = sel.any(axis=1)
        tok = np.nonzero(any_e)[0]
        ge = np.where(sel[tok, 0], w[tok, 0], w[tok, 1])
        tok_lists.append(tok)
        gate_lists.append(ge.astype(np.float64))

    maxT = max(len(t) for t in tok_lists)
    C = max(256, maxT)

    nc, rsz, ssz = _get_program(C)
    rofs = np.cumsum([0] + rsz)
    sofs = np.cumsum([0] + ssz)

    # --- per-core input maps ---
    w1s = [_pack_w(ew1[e], 16) for e in range(E)]
    w2s = [_pack_w(ew2[e], 4) for e in range(E)]
    w3s = [_pack_w(ew3[e], 2) for e in range(E)]
    s1 = _pack_w(sw1, 2)
    s2 = _pack_w(sw2, 4)
    s3 = _pack_w(sw3, 2)

    in_maps = []
    for e in range(E):
        tok = tok_lists[e]
        xg = np.zeros((C, D), dtype=np.float32)
        xg[: len(tok)] = xf[tok]
        m = {}
        for j, blk in enumerate(w1s[e]):
            m[f"w1_{j}"] = blk
        for j, blk in enumerate(w2s[e]):
            m[f"w2_{j}"] = blk
        for j, blk in enumerate(w3s[e]):
            m[f"w3_{j}"] = blk
        for j, blk in enumerate(s1):
            m[f"s1_{j}"] = blk
        for j, blk in enumerate(s2):
            m[f"s2_{j}"] = blk
        for j, blk in enumerate(s3):
            m[f"s3_{j}"] = blk
        for i, sz in enumerate(rsz):
            m[f"xg{i}"] = _pack_x_chunk(xg[rofs[i] : rofs[i] + sz], pad=(i > 0))
        xs = xf[e * S : (e + 1) * S]
        for i, sz in enumerate(ssz):
            m[f"xs{i}"] = _pack_x_chunk(xs[sofs[i] : sofs[i] + sz], pad=True)
        in_maps.append(m)

    res = run_bass_kernel_spmd(nc, in_maps, list(range(E)))

    # --- host combine: shared shards + gated scatter-add of routed outputs ---
    out = np.empty((N_TOK, D), dtype=np.float32)
    for e in range(E):
        r = res.results[e]
        out[e * S : (e + 1) * S] = _unpack_out(
            [r[f"os{i}"] for i in range(len(ssz))]
        )

    all_tok = np.concatenate(tok_lists)
    all_contrib = np.concatenate(
        [
            _unpack_out([res.results[e][f"or{i}"] for i in range(len(rsz))])[
                : len(tok_lists[e])
            ]
            * gate_lists[e][:, None].astype(np.float32)
            for e in range(E)
        ]
    )
    pos = np.argsort(all_tok, kind="stable")
    # every token has exactly two routed contributions (top-2 routing)
    out += all_contrib[pos[0::2]]
    out += all_contrib[pos[1::2]]

    return out.reshape(x.shape).astype(np.float32)

